# revision 1
# baseline (speedup 1.0000x reference)
"""Chunked gated-linear-attention (GLA) kernel for Trainium2, 8 NeuronCores.

Math (per (b,h), per-head scalar decay lam):
    S_t = lam * S_{t-1} + k_t^T v_t ;  o_t = (q_t * SCALE) @ S_t

Block-parallel form, chunk C=128, state updated every PAIR of chunks
(stride 256) to halve the serial state-chain depth:
    pair (c0, c1):
      W00[j,i] = k_j.q_i (both in c0) * SCALE*lam^(i-j) * [j<=i]
      W11      = same within c1
      WX [j,i] = k_j (c0) . q_i (c1) * SCALE*lam^(128+i-j)   (dense)
      O(c0)[i] = sum_j W00[j,i] V0[j] + SCALE*lam^(i+1)   q_i . S
      O(c1)[i] = sum_j W11[j,i] V1[j] + sum_j WX[j,i] V0[j]
                 + SCALE*lam^(128+i+1) q_i . S
      S <- lam^256 S + sum_j lam^(255-j') k_j' v_j'   (j' pair-relative)

Sharding: B*H = 32 (b,h) units, 4 per core (head-parallel, no collectives).
Host prep (part of sharding): cast to fp16, pre-transpose Q/K to [D,T],
pack K|V rows so natural-layout DMA descriptors are 512B.
All matmul operands fp16 (PSUM accumulates fp32); measured rel_l2 ~5e-4.
"""

import math
from contextlib import ExitStack

import numpy as np

import concourse.bacc as bacc
import concourse.mybir as mybir
import concourse.tile as tile
from concourse.bass_utils import run_bass_kernel_spmd

B, T, H, D = 2, 2048, 16, 128
C = 128                  # chunk size along time
NCH = T // C             # 16 chunks
G = 4                    # chunks per load group
NG = NCH // G            # 4 groups
GC = G * C               # 512
NCORES = 8
U = (B * H) // NCORES    # 4 (b,h) units per core
SCALE = 0.08838834764831845
LAYER_IDX, NUM_LAYERS = 12, 32

F32 = mybir.dt.float32
F16 = mybir.dt.float16

TRACE = False            # test.py sets True to capture an NTFF profile
LAST = {}


def _slopes(n):
    def p2(m):
        start = 2.0 ** (-(2.0 ** (-(math.log2(m) - 3))))
        return [start * start**i for i in range(m)]

    if math.log2(n).is_integer():
        return p2(n)
    cp = 2 ** math.floor(math.log2(n))
    return p2(cp) + _slopes(2 * cp)[0::2][: n - cp]


def _lambdas():
    s = -np.asarray(_slopes(H), dtype=np.float64) * (
        1.0 - LAYER_IDX / (NUM_LAYERS - 1) + 1e-5
    )
    return np.exp(s)


def _build_nc():
    nc = bacc.Bacc(trn_type="TRN2", debug=False, num_devices=NCORES)

    qt = nc.dram_tensor("qt", [U, D, T], F16, kind="ExternalInput")
    kt = nc.dram_tensor("kt", [U, D, T], F16, kind="ExternalInput")
    kv = nc.dram_tensor("kv", [U, T, 2 * D], F16, kind="ExternalInput")
    s0 = nc.dram_tensor("s0", [U, D, D], F16, kind="ExternalInput")
    # maskc[j, u*C+i] = SCALE*lam_u^(i-j) for i>=j else 0   (within-chunk)
    maskc = nc.dram_tensor("maskc", [128, U * C], F16, kind="ExternalInput")
    # maskx[j, u*C+i] = SCALE*lam_u^(128+i-j)               (cross-chunk, dense)
    maskx = nc.dram_tensor("maskx", [128, U * C], F16, kind="ExternalInput")
    # sdg[:, u*D:(u+1)*D] = lam_u^256 * I
    sdg = nc.dram_tensor("sdg", [128, U * D], F16, kind="ExternalInput")
    # qdm[d, u*GC + cc*C + i] = SCALE*lam_u^((cc%2)*128 + i + 1)
    qdm = nc.dram_tensor("qdm", [128, U * GC], F16, kind="ExternalInput")
    # ckm[j, u*GC + cc*C + d] = lam_u^((255 if cc%2==0 else 127) - j)
    ckm = nc.dram_tensor("ckm", [128, U * GC], F16, kind="ExternalInput")
    o = nc.dram_tensor("o", [U, T, D], F32, kind="ExternalOutput")

    with tile.TileContext(nc) as tc, ExitStack() as ctx:
        const = ctx.enter_context(tc.tile_pool(name="const", bufs=1))
        h16 = ctx.enter_context(tc.tile_pool(name="h16", bufs=2))
        outp = ctx.enter_context(tc.tile_pool(name="outp", bufs=4))
        psum = ctx.enter_context(tc.tile_pool(name="psum", bufs=2, space="PSUM"))
        state = ctx.enter_context(tc.tile_pool(name="state", bufs=2))

        def load_group(g):
            t0 = g * GC
            qtb = h16.tile([128, U * GC], F16, tag="qtb", bufs=3, name=f"qtb{g}")
            nc.sync.dma_start(
                qtb[:].rearrange("p (u t) -> p u t", u=U),
                qt[:, :, t0 : t0 + GC].rearrange("u d t -> d u t"),
            )
            ktb = h16.tile([128, U * GC], F16, tag="ktb", bufs=3, name=f"ktb{g}")
            nc.sync.dma_start(
                ktb[:].rearrange("p (u t) -> p u t", u=U),
                kt[:, :, t0 : t0 + GC].rearrange("u d t -> d u t"),
            )
            kvb = h16.tile(
                [128, U * G * 2 * D], F16, tag="kvb", bufs=3, name=f"kvb{g}"
            )
            for u in range(U):
                nc.sync.dma_start(
                    kvb[:, u * G * 2 * D : (u + 1) * G * 2 * D].rearrange(
                        "p (c x) -> p c x", c=G
                    ),
                    kv[u, t0 : t0 + GC, :].rearrange("(c p) x -> p c x", p=128),
                )
            return qtb, ktb, kvb

        # group-0 loads first so the big DMAs start immediately
        g0_tiles = load_group(0)

        mask_sb = const.tile([128, U * C], F16)
        nc.sync.dma_start(mask_sb[:], maskc[:])
        maskx_sb = const.tile([128, U * C], F16)
        nc.sync.dma_start(maskx_sb[:], maskx[:])
        sdg_sb = const.tile([128, U * D], F16)
        nc.sync.dma_start(sdg_sb[:], sdg[:])
        qdm_sb = const.tile([128, U * GC], F16)
        nc.sync.dma_start(qdm_sb[:], qdm[:])
        ckm_sb = const.tile([128, U * GC], F16)
        nc.sync.dma_start(ckm_sb[:], ckm[:])

        s_cur = state.tile([128, U * D], F16, tag="ssb")
        nc.sync.dma_start(
            s_cur[:].rearrange("p (u x) -> p u x", u=U),
            s0[:].rearrange("u d x -> d u x"),
        )

        for g in range(NG):
            qtb, ktb, kvb = g0_tiles if g == 0 else load_group(g)
            kvv = kvb[:].rearrange("p (u c x d) -> p u c x d", u=U, c=G, x=2)

            qdec, kd = {}, {}
            for u in range(U):
                us = slice(u * GC, (u + 1) * GC)
                qdec_t = h16.tile([128, GC], F16, tag="qdec", bufs=8)
                nc.gpsimd.tensor_tensor(
                    qdec_t[:], qtb[:, us], qdm_sb[:, us], mybir.AluOpType.mult
                )
                kd_t = h16.tile([128, GC], F16, tag="kd", bufs=8)
                nc.vector.tensor_tensor(
                    kd_t[:].rearrange("p (c d) -> p c d", c=G),
                    kvv[:, u, :, 0, :],
                    ckm_sb[:, us].rearrange("p (c d) -> p c d", c=G),
                    mybir.AluOpType.mult,
                )
                qdec[u], kd[u] = qdec_t, kd_t

            def wslice(u, cc):
                return slice(u * GC + cc * C, u * GC + (cc + 1) * C)

            for pp in range(G // 2):
                cc0, cc1 = 2 * pp, 2 * pp + 1
                c0 = g * G + cc0

                # --- chunk c0 ---
                w0 = psum.tile([128, U * C], F32, tag="w", bufs=3)
                for u in range(U):
                    nc.tensor.matmul(
                        w0[:, u * C : (u + 1) * C], lhsT=ktb[:, wslice(u, cc0)],
                        rhs=qtb[:, wslice(u, cc0)], start=True, stop=True,
                    )
                wm0 = h16.tile([128, U * C], F16, tag="wm", bufs=6)
                nc.vector.tensor_tensor(
                    wm0[:], w0[:], mask_sb[:], mybir.AluOpType.mult
                )
                o0 = psum.tile([128, U * D], F32, tag="o")
                for u in range(U):
                    ds = slice(u * D, (u + 1) * D)
                    v0 = kvv[:, u, cc0, 1, :]
                    nc.tensor.matmul(
                        o0[:, ds], lhsT=wm0[:, u * C : (u + 1) * C],
                        rhs=v0, start=True, stop=False,
                    )
                    nc.tensor.matmul(
                        o0[:, ds], lhsT=qdec[u][:, cc0 * C : (cc0 + 1) * C],
                        rhs=s_cur[:, ds], start=False, stop=True,
                    )
                ob0 = outp.tile([128, U * D], F32, tag="ob")
                nc.scalar.copy(ob0[:], o0[:])
                nc.scalar.dma_start(
                    o[:, c0 * C : (c0 + 1) * C, :].rearrange("u p d -> p u d"),
                    ob0[:].rearrange("p (u d) -> p u d", u=U),
                )

                # --- chunk c1 ---
                wx = psum.tile([128, U * C], F32, tag="w", bufs=3)
                for u in range(U):
                    nc.tensor.matmul(
                        wx[:, u * C : (u + 1) * C], lhsT=ktb[:, wslice(u, cc0)],
                        rhs=qtb[:, wslice(u, cc1)], start=True, stop=True,
                    )
                wmx = h16.tile([128, U * C], F16, tag="wm", bufs=6)
                nc.vector.tensor_tensor(
                    wmx[:], wx[:], maskx_sb[:], mybir.AluOpType.mult
                )
                w1 = psum.tile([128, U * C], F32, tag="w", bufs=3)
                for u in range(U):
                    nc.tensor.matmul(
                        w1[:, u * C : (u + 1) * C], lhsT=ktb[:, wslice(u, cc1)],
                        rhs=qtb[:, wslice(u, cc1)], start=True, stop=True,
                    )
                wm1 = h16.tile([128, U * C], F16, tag="wm", bufs=6)
                nc.vector.tensor_tensor(
                    wm1[:], w1[:], mask_sb[:], mybir.AluOpType.mult
                )
                o1 = psum.tile([128, U * D], F32, tag="o")
                for u in range(U):
                    ds = slice(u * D, (u + 1) * D)
                    v0 = kvv[:, u, cc0, 1, :]
                    v1 = kvv[:, u, cc1, 1, :]
                    nc.tensor.matmul(
                        o1[:, ds], lhsT=wm1[:, u * C : (u + 1) * C],
                        rhs=v1, start=True, stop=False,
                    )
                    nc.tensor.matmul(
                        o1[:, ds], lhsT=wmx[:, u * C : (u + 1) * C],
                        rhs=v0, start=False, stop=False,
                    )
                    nc.tensor.matmul(
                        o1[:, ds], lhsT=qdec[u][:, cc1 * C : (cc1 + 1) * C],
                        rhs=s_cur[:, ds], start=False, stop=True,
                    )
                ob1 = outp.tile([128, U * D], F32, tag="ob")
                nc.scalar.copy(ob1[:], o1[:])
                nc.scalar.dma_start(
                    o[:, (c0 + 1) * C : (c0 + 2) * C, :].rearrange(
                        "u p d -> p u d"
                    ),
                    ob1[:].rearrange("p (u d) -> p u d", u=U),
                )

                # --- state update (once per pair) ---
                s_bank = psum.tile([128, U * D], F32, tag="s")
                for u in range(U):
                    ds = slice(u * D, (u + 1) * D)
                    nc.tensor.matmul(
                        s_bank[:, ds], lhsT=sdg_sb[:, ds],
                        rhs=s_cur[:, ds], start=True, stop=False,
                    )
                    nc.tensor.matmul(
                        s_bank[:, ds], lhsT=kd[u][:, cc0 * C : (cc0 + 1) * C],
                        rhs=kvv[:, u, cc0, 1, :], start=False, stop=False,
                    )
                    nc.tensor.matmul(
                        s_bank[:, ds], lhsT=kd[u][:, cc1 * C : (cc1 + 1) * C],
                        rhs=kvv[:, u, cc1, 1, :], start=False, stop=True,
                    )
                s_new = state.tile([128, U * D], F16, tag="ssb")
                half = U * D // 2
                nc.scalar.copy(s_new[:, 0:half], s_bank[:, 0:half])
                nc.vector.tensor_copy(s_new[:, half:], s_bank[:, half:])
                s_cur = s_new

    nc.compile()
    return nc


_NC_CACHE = []


def _get_nc():
    if not _NC_CACHE:
        _NC_CACHE.append(_build_nc())
    return _NC_CACHE[0]


def _core_consts(core):
    lam = _lambdas()
    i_idx = np.arange(C).astype(np.float64)
    maskc = np.zeros((128, U * C), np.float16)
    maskx = np.zeros((128, U * C), np.float16)
    sdg = np.zeros((128, U * D), np.float16)
    qdm = np.zeros((128, U * GC), np.float16)
    ckm = np.zeros((128, U * GC), np.float16)
    eye = np.eye(128, dtype=np.float64)
    for u in range(U):
        h = (U * core + u) % H
        l = lam[h]
        m = np.where(
            i_idx[None, :] >= i_idx[:, None],
            SCALE * l ** (i_idx[None, :] - i_idx[:, None]),
            0.0,
        )
        maskc[:, u * C : (u + 1) * C] = m.astype(np.float16)
        mx = SCALE * l ** (128.0 + i_idx[None, :] - i_idx[:, None])
        maskx[:, u * C : (u + 1) * C] = mx.astype(np.float16)
        sdg[:, u * D : (u + 1) * D] = (l ** 256 * eye).astype(np.float16)
        for cc in range(G):
            par = cc % 2
            cq = (SCALE * l ** (par * 128 + i_idx + 1)).astype(np.float16)
            qdm[:, u * GC + cc * C : u * GC + (cc + 1) * C] = np.tile(
                cq, (128, 1)
            )
            ck = (l ** ((255.0 if par == 0 else 127.0) - i_idx)).astype(
                np.float16
            )
            ckm[:, u * GC + cc * C : u * GC + (cc + 1) * C] = np.repeat(
                ck[:, None], C, axis=1
            )
    return maskc, maskx, sdg, qdm, ckm


def kernel(query_states, key_states, value_states, initial_state):
    q16 = np.asarray(query_states).astype(np.float16)
    k16 = np.asarray(key_states).astype(np.float16)
    v16 = np.asarray(value_states).astype(np.float16)
    # [B,T,H,D] -> [B*H, T, D]
    q16 = np.transpose(q16, (0, 2, 1, 3)).reshape(B * H, T, D)
    k16 = np.transpose(k16, (0, 2, 1, 3)).reshape(B * H, T, D)
    v16 = np.transpose(v16, (0, 2, 1, 3)).reshape(B * H, T, D)
    s016 = np.asarray(initial_state).astype(np.float16).reshape(B * H, D, D)

    nc = _get_nc()
    in_maps = []
    for core in range(NCORES):
        lo = U * core
        maskc, maskx, sdg, qdm, ckm = _core_consts(core)
        in_maps.append(
            {
                "qt": np.ascontiguousarray(q16[lo : lo + U].transpose(0, 2, 1)),
                "kt": np.ascontiguousarray(k16[lo : lo + U].transpose(0, 2, 1)),
                "kv": np.ascontiguousarray(
                    np.concatenate([k16[lo : lo + U], v16[lo : lo + U]], axis=2)
                ),
                "s0": np.ascontiguousarray(s016[lo : lo + U]),
                "maskc": maskc,
                "maskx": maskx,
                "sdg": sdg,
                "qdm": qdm,
                "ckm": ckm,
            }
        )

    res = run_bass_kernel_spmd(
        nc, in_maps, core_ids=list(range(NCORES)), trace=TRACE
    )
    if TRACE:
        LAST["exec_time_ns"] = res.exec_time_ns
        LAST["mean_exec_time_ns"] = res.mean_exec_time_ns
        LAST["trace"] = (
            res.instructions_and_trace[1] if res.instructions_and_trace else None
        )

    out = np.empty((B * H, T, D), np.float32)
    for core in range(NCORES):
        out[U * core : U * core + U] = res.results[core]["o"]
    return np.ascontiguousarray(
        np.transpose(out.reshape(B, H, T, D), (0, 2, 1, 3))
    )



# revision 9
# speedup vs baseline: 1.0600x; 1.0600x over previous
"""Chunked gated-linear-attention (GLA) kernel for Trainium2, 8 NeuronCores.

Math (per (b,h), per-head scalar decay lam):
    S_t = lam * S_{t-1} + k_t^T v_t ;  o_t = (q_t * SCALE) @ S_t

Block form, chunk C=128, state updated every chunk:
    chunk c:
      W[j,i]  = k_j.q_i * SCALE*lam^(i-j) * [j<=i]        (within-chunk)
      O[i]    = sum_j W[j,i] V[j] + SCALE*lam^(i+1) q_i . S
      S      <- lam^128 S + sum_j lam^(127-j) k_j v_j

Sharding: B*H = 32 (b,h) units, 4 per core (head-parallel, no collectives).
Host prep: cast fp16, pack Q pre-transposed [D,T] and K|V natural [T,2D]
into per-block (2-chunk) contiguous layouts so every DMA descriptor is
2-4KB per partition. K is loaded once; K^T for the W matmul is produced
on-chip via tensor-engine transposes. Output is written fp16 (host
upcasts to fp32). All matmuls fp16 -> PSUM fp32.
"""

import math
from contextlib import ExitStack

import numpy as np

import concourse.bacc as bacc
import concourse.mybir as mybir
import concourse.tile as tile
from concourse.bass_utils import run_bass_kernel_spmd

B, T, H, D = 2, 2048, 16, 128
C = 128                  # chunk size along time
NCH = T // C             # 16 chunks
BC = 2                   # chunks per DMA block
NB = NCH // BC           # 8 blocks
NCORES = 8
U = (B * H) // NCORES    # 4 (b,h) units per core
SCALE = 0.08838834764831845
LAYER_IDX, NUM_LAYERS = 12, 32

F32 = mybir.dt.float32
F16 = mybir.dt.float16

# const tensor column offsets: [maskc(U*C) | qdm(U*C) | ident(C)]
CST_MASK = 0
CST_QDM = U * C
CST_ID = 2 * U * C
CST_W = 2 * U * C + C

TRACE = False            # test.py sets True to capture an NTFF profile
LAST = {}


def _slopes(n):
    def p2(m):
        start = 2.0 ** (-(2.0 ** (-(math.log2(m) - 3))))
        return [start * start**i for i in range(m)]

    if math.log2(n).is_integer():
        return p2(n)
    cp = 2 ** math.floor(math.log2(n))
    return p2(cp) + _slopes(2 * cp)[0::2][: n - cp]


def _lambdas():
    s = -np.asarray(_slopes(H), dtype=np.float64) * (
        1.0 - LAYER_IDX / (NUM_LAYERS - 1) + 1e-5
    )
    return np.exp(s)


def _build_nc():
    nc = bacc.Bacc(trn_type="TRN2", debug=False, num_devices=NCORES)

    # qt[b, d, (cc, u, i)] : Q pre-transposed, per 2-chunk block
    qt = nc.dram_tensor("qt", [NB, 128, BC * U * C], F16, kind="ExternalInput")
    # kv[b, p, (cc, u, x, d)] : K|V natural layout, per 2-chunk block
    kv = nc.dram_tensor("kv", [NB, 128, BC * U * 2 * D], F16, kind="ExternalInput")
    # s0[dk, (u, dv)]
    s0 = nc.dram_tensor("s0", [128, U * D], F16, kind="ExternalInput")
    # cst[:, :U*C] = maskc: SCALE*lam_u^(i-j) causal; [U*C:2U*C] = qdm:
    # SCALE*lam_u^(i+1) (bcast over partitions); [2U*C:2U*C+C] = identity
    cst = nc.dram_tensor("cst", [128, CST_W], F16, kind="ExternalInput")
    # ckf[j, u] = lam_u^(127-j); ckf[j, U+u] = lam_u^128 (fp32)
    ckf = nc.dram_tensor("ckf", [128, 2 * U], F32, kind="ExternalInput")
    # o[b, i, (cc, u, dv)] fp16
    o = nc.dram_tensor("o", [NB, 128, BC * U * D], F16, kind="ExternalOutput")

    lam = _lambdas()

    with tile.TileContext(nc) as tc, ExitStack() as ctx:
        const = ctx.enter_context(tc.tile_pool(name="const", bufs=1))
        ld = ctx.enter_context(tc.tile_pool(name="ld", bufs=1))
        h16 = ctx.enter_context(tc.tile_pool(name="h16", bufs=3))
        outp = ctx.enter_context(tc.tile_pool(name="outp", bufs=3))
        state = ctx.enter_context(tc.tile_pool(name="state", bufs=2))
        psum = ctx.enter_context(tc.tile_pool(name="psum", bufs=2, space="PSUM"))

        # small consts on the ACT (scalar) DMA ring; bulk loads on SP (sync)
        cst_sb = const.tile([128, CST_W], F16)
        nc.scalar.dma_start(cst_sb[:], cst[:])
        ckf_sb = const.tile([128, 2 * U], F32)
        nc.scalar.dma_start(ckf_sb[:], ckf[:])
        s_cur = state.tile([128, U * D], F16, tag="ssb")
        nc.scalar.dma_start(s_cur[:], s0[:])

        mask_sb = cst_sb[:, CST_MASK:CST_QDM]
        qdm_sb = cst_sb[:, CST_QDM:CST_ID]
        ident = cst_sb[:, CST_ID:CST_W]

        # stream in all blocks up-front; SDMA drains the ring in FIFO order
        qtiles, kvtiles = [], []
        for b in range(NB):
            qb = ld.tile([128, BC * U * C], F16, tag="qb", bufs=NB, name=f"qb{b}")
            nc.sync.dma_start(qb[:], qt[b])
            kvb = ld.tile(
                [128, BC * U * 2 * D], F16, tag="kvb", bufs=NB, name=f"kvb{b}"
            )
            nc.sync.dma_start(kvb[:], kv[b])
            qtiles.append(qb)
            kvtiles.append(kvb)

        for b in range(NB):
            qbv = qtiles[b][:].rearrange("p (cc u i) -> p cc u i", cc=BC, u=U)
            kvv = kvtiles[b][:].rearrange(
                "p (cc u x d) -> p cc u x d", cc=BC, u=U, x=2
            )
            ob = outp.tile([128, BC * U * D], F16, tag="ob")

            for cc in range(BC):
                # K^T for this chunk via tensor-engine transpose
                pkt = psum.tile([128, U * C], F16, tag="kt")
                for u in range(U):
                    nc.tensor.transpose(
                        pkt[:, u * C : (u + 1) * C], kvv[:, cc, u, 0, :], ident
                    )
                ktb = h16.tile([128, U * C], F16, tag="ktb")
                nc.scalar.copy(ktb[:], pkt[:])

                # qdec = q * SCALE*lam^(i+1)  (gpsimd, independent of psum)
                qdec = h16.tile([128, U * C], F16, tag="qdec")
                nc.gpsimd.tensor_tensor(
                    qdec[:].rearrange("p (u i) -> p u i", u=U),
                    qbv[:, cc],
                    qdm_sb[:].rearrange("p (u i) -> p u i", u=U),
                    mybir.AluOpType.mult,
                )

                # kd[j, (u,d)] = k * lam_u^(127-j)
                kd = h16.tile([128, U * D], F16, tag="kd")
                for u in range(U):
                    nc.vector.tensor_scalar(
                        kd[:, u * D : (u + 1) * D],
                        kvv[:, cc, u, 0, :],
                        ckf_sb[:, u : u + 1],
                        None,
                        mybir.AluOpType.mult,
                    )

                # W = K^T Q, then mask
                pw = psum.tile([128, U * C], F32, tag="w")
                for u in range(U):
                    nc.tensor.matmul(
                        pw[:, u * C : (u + 1) * C],
                        lhsT=ktb[:, u * C : (u + 1) * C],
                        rhs=qbv[:, cc, u, :],
                        start=True,
                        stop=True,
                    )
                wm = h16.tile([128, U * C], F16, tag="wm")
                nc.vector.tensor_tensor(
                    wm[:], pw[:], mask_sb[:], mybir.AluOpType.mult
                )

                # O = Wm^T V + qdec^T S
                po = psum.tile([128, U * D], F32, tag="o")
                for u in range(U):
                    ds = slice(u * D, (u + 1) * D)
                    nc.tensor.matmul(
                        po[:, ds],
                        lhsT=wm[:, u * C : (u + 1) * C],
                        rhs=kvv[:, cc, u, 1, :],
                        start=True,
                        stop=False,
                    )
                    nc.tensor.matmul(
                        po[:, ds],
                        lhsT=qdec[:, u * C : (u + 1) * C],
                        rhs=s_cur[:, ds],
                        start=False,
                        stop=True,
                    )

                # state KV update, then fused decay+add on DVE
                ps = psum.tile([128, U * D], F32, tag="s")
                for u in range(U):
                    ds = slice(u * D, (u + 1) * D)
                    nc.tensor.matmul(
                        ps[:, ds],
                        lhsT=kd[:, ds],
                        rhs=kvv[:, cc, u, 1, :],
                        start=True,
                        stop=True,
                    )
                s_new = state.tile([128, U * D], F16, tag="ssb")
                for u in range(U):
                    ds = slice(u * D, (u + 1) * D)
                    nc.vector.scalar_tensor_tensor(
                        s_new[:, ds],
                        s_cur[:, ds],
                        ckf_sb[:, U + u : U + u + 1],
                        ps[:, ds],
                        mybir.AluOpType.mult,
                        mybir.AluOpType.add,
                    )
                s_cur = s_new

                nc.scalar.copy(ob[:, cc * U * D : (cc + 1) * U * D], po[:])

            nc.scalar.dma_start(o[b], ob[:])

    nc.compile()
    return nc


_NC_CACHE = []


def _get_nc():
    if not _NC_CACHE:
        _NC_CACHE.append(_build_nc())
    return _NC_CACHE[0]


def _core_consts(core):
    lam = _lambdas()
    i_idx = np.arange(C).astype(np.float64)
    cstv = np.zeros((128, CST_W), np.float16)
    ckfv = np.zeros((128, 2 * U), np.float32)
    for u in range(U):
        h = (U * core + u) % H
        l = lam[h]
        m = np.where(
            i_idx[None, :] >= i_idx[:, None],
            SCALE * l ** (i_idx[None, :] - i_idx[:, None]),
            0.0,
        )
        cstv[:, CST_MASK + u * C : CST_MASK + (u + 1) * C] = m.astype(np.float16)
        cq = (SCALE * l ** (i_idx + 1)).astype(np.float16)
        cstv[:, CST_QDM + u * C : CST_QDM + (u + 1) * C] = np.tile(cq, (128, 1))
        ckfv[:, u] = (l ** (127.0 - i_idx)).astype(np.float32)
        ckfv[:, U + u] = np.float32(l ** C)
    cstv[:, CST_ID : CST_W] = np.eye(128, dtype=np.float16)
    return cstv, ckfv


def kernel(query_states, key_states, value_states, initial_state):
    q16 = np.asarray(query_states).astype(np.float16)
    k16 = np.asarray(key_states).astype(np.float16)
    v16 = np.asarray(value_states).astype(np.float16)
    # [B,T,H,D] -> [B*H, T, D]
    q16 = np.transpose(q16, (0, 2, 1, 3)).reshape(B * H, T, D)
    k16 = np.transpose(k16, (0, 2, 1, 3)).reshape(B * H, T, D)
    v16 = np.transpose(v16, (0, 2, 1, 3)).reshape(B * H, T, D)
    s016 = np.asarray(initial_state).astype(np.float16).reshape(B * H, D, D)

    nc = _get_nc()
    in_maps = []
    for core in range(NCORES):
        lo = U * core
        cstv, ckfv = _core_consts(core)
        qs = q16[lo : lo + U]  # [U, T, D]
        ks = k16[lo : lo + U]
        vs = v16[lo : lo + U]
        # qt[b, d, (cc,u,i)]
        qb = qs.reshape(U, NB, BC, C, D).transpose(1, 4, 2, 0, 3)
        qb = np.ascontiguousarray(qb.reshape(NB, 128, BC * U * C))
        # kv[b, p, (cc,u,x,d)]
        kvb = np.stack(
            [ks.reshape(U, NB, BC, C, D), vs.reshape(U, NB, BC, C, D)], axis=4
        )  # [U, NB, BC, C, 2, D]
        kvb = kvb.transpose(1, 3, 2, 0, 4, 5)  # [NB, C, BC, U, 2, D]
        kvb = np.ascontiguousarray(kvb.reshape(NB, 128, BC * U * 2 * D))
        s0b = np.ascontiguousarray(
            s016[lo : lo + U].transpose(1, 0, 2).reshape(128, U * D)
        )
        in_maps.append(
            {"qt": qb, "kv": kvb, "s0": s0b, "cst": cstv, "ckf": ckfv}
        )

    res = run_bass_kernel_spmd(
        nc, in_maps, core_ids=list(range(NCORES)), trace=TRACE
    )
    if TRACE:
        LAST["exec_time_ns"] = res.exec_time_ns
        LAST["mean_exec_time_ns"] = res.mean_exec_time_ns
        LAST["trace"] = (
            res.instructions_and_trace[1] if res.instructions_and_trace else None
        )

    out = np.empty((B * H, T, D), np.float32)
    for core in range(NCORES):
        ob = res.results[core]["o"].reshape(NB, C, BC, U, D)
        # -> [U, NB, BC, C, D] -> [U, T, D]
        out[U * core : U * core + U] = (
            ob.transpose(3, 0, 2, 1, 4).reshape(U, T, D).astype(np.float32)
        )
    return np.ascontiguousarray(
        np.transpose(out.reshape(B, H, T, D), (0, 2, 1, 3))
    )


# revision 19
# speedup vs baseline: 1.3556x; 1.2788x over previous
"""Chunked gated-linear-attention (GLA) kernel for Trainium2, 8 NeuronCores.

Math (per (b,h), per-head scalar decay lam):
    S_t = lam * S_{t-1} + k_t^T v_t ;  o_t = (q_t * SCALE) @ S_t

Block form, chunk C=128, state updated every chunk:
    chunk c:
      W[j,i]  = k_j.q_i * SCALE*lam^(i-j) * [j<=i]        (within-chunk)
      O[i]    = sum_j W[j,i] V[j] + SCALE*lam^(i+1) q_i . S
      S      <- lam^128 S + sum_j lam^(127-j) k_j v_j

Sharding: B*H = 32 (b,h) units, 4 per core (head-parallel, no collectives).
Host prep: cast fp16, pack Q pre-transposed [D,T] and K|V natural [T,2D]
into per-block (2-chunk) contiguous layouts so every DMA descriptor is
2-4KB per partition. K is loaded once; K^T for the W matmul is produced
on-chip via tensor-engine transposes. Output is written fp16 (host
upcasts to fp32). All matmuls fp16 -> PSUM fp32.
"""

import math
from contextlib import ExitStack

import numpy as np

import concourse.bacc as bacc
import concourse.mybir as mybir
import concourse.tile as tile
from concourse.bass_utils import run_bass_kernel_spmd

B, T, H, D = 2, 2048, 16, 128
C = 128                  # chunk size along time
NCH = T // C             # 16 chunks
BC = 2                   # chunks per DMA block
NB = NCH // BC           # 8 blocks
NCORES = 8
U = (B * H) // NCORES    # 4 (b,h) units per core
SCALE = 0.08838834764831845
LAYER_IDX, NUM_LAYERS = 12, 32

F32 = mybir.dt.float32
F16 = mybir.dt.float16

# const tensor column offsets: [maskc | qdm | ckm | sdg | ident]
CST_MASK = 0
CST_QDM = U * C
CST_CKM = 2 * U * C
CST_SDG = 3 * U * C
CST_ID = 4 * U * C
CST_W = 4 * U * C + C

TRACE = False            # test.py sets True to capture an NTFF profile
LAST = {}


def _slopes(n):
    def p2(m):
        start = 2.0 ** (-(2.0 ** (-(math.log2(m) - 3))))
        return [start * start**i for i in range(m)]

    if math.log2(n).is_integer():
        return p2(n)
    cp = 2 ** math.floor(math.log2(n))
    return p2(cp) + _slopes(2 * cp)[0::2][: n - cp]


def _lambdas():
    s = -np.asarray(_slopes(H), dtype=np.float64) * (
        1.0 - LAYER_IDX / (NUM_LAYERS - 1) + 1e-5
    )
    return np.exp(s)


def _build_nc():
    nc = bacc.Bacc(trn_type="TRN2", debug=False, num_devices=NCORES)

    # qt[b, d, (cc, u, i)] : Q pre-transposed, per 2-chunk block
    qt = nc.dram_tensor("qt", [NB, 128, BC * U * C], F16, kind="ExternalInput")
    # kv[b, p, (cc, u, x, d)] : K|V natural layout, per 2-chunk block
    kv = nc.dram_tensor("kv", [NB, 128, BC * U * 2 * D], F16, kind="ExternalInput")
    # s0[dk, (u, dv)]
    s0 = nc.dram_tensor("s0", [128, U * D], F16, kind="ExternalInput")
    # cst: maskc = SCALE*lam_u^(i-j) causal; qdm = SCALE*lam_u^(i+1)
    # (bcast over partitions); ckm[j, (u,d)] = lam_u^(127-j);
    # sdg[dk, (u,dk')] = lam_u^128 * I; ident
    cst = nc.dram_tensor("cst", [128, CST_W], F16, kind="ExternalInput")
    # o[b2, i, (cc4, u, dv)] fp16, 4 chunks per store block
    o = nc.dram_tensor("o", [NB // 2, 128, 2 * BC * U * D], F16, kind="ExternalOutput")

    with tile.TileContext(nc) as tc, ExitStack() as ctx:
        const = ctx.enter_context(tc.tile_pool(name="const", bufs=1))
        ld = ctx.enter_context(tc.tile_pool(name="ld", bufs=1))
        h16 = ctx.enter_context(tc.tile_pool(name="h16", bufs=3))
        outp = ctx.enter_context(tc.tile_pool(name="outp", bufs=3))
        state = ctx.enter_context(tc.tile_pool(name="state", bufs=2))
        psum = ctx.enter_context(tc.tile_pool(name="psum", bufs=2, space="PSUM"))

        # small consts on the ACT (scalar) DMA ring; bulk loads on SP (sync)
        cst_sb = const.tile([128, CST_W], F16)
        nc.scalar.dma_start(cst_sb[:], cst[:])
        s_cur = state.tile([128, U * D], F16, tag="ssb")
        nc.scalar.dma_start(s_cur[:], s0[:])

        mask_sb = cst_sb[:, CST_MASK:CST_QDM]
        qdm_sb = cst_sb[:, CST_QDM:CST_CKM]
        ckm_sb = cst_sb[:, CST_CKM:CST_SDG]
        sdg_sb = cst_sb[:, CST_SDG:CST_ID]
        ident = cst_sb[:, CST_ID:CST_W]

        # stream in all blocks up-front; SDMA drains the ring in FIFO order
        qtiles, kvtiles = [], []
        for b in range(NB):
            qb = ld.tile([128, BC * U * C], F16, tag="qb", bufs=NB, name=f"qb{b}")
            nc.sync.dma_start(qb[:], qt[b])
            kvb = ld.tile(
                [128, BC * U * 2 * D], F16, tag="kvb", bufs=NB, name=f"kvb{b}"
            )
            nc.sync.dma_start(kvb[:], kv[b])
            qtiles.append(qb)
            kvtiles.append(kvb)

        for b in range(NB):
            qbv = qtiles[b][:].rearrange("p (cc u i) -> p cc u i", cc=BC, u=U)
            kvv = kvtiles[b][:].rearrange(
                "p (cc u x d) -> p cc u x d", cc=BC, u=U, x=2
            )
            if b % 2 == 0:
                ob = outp.tile([128, 2 * BC * U * D], F16, tag="ob")

            for cc in range(BC):
                # K^T for this chunk via tensor-engine transpose
                pkt = psum.tile([128, U * C], F16, tag="kt")
                for u in range(U):
                    nc.tensor.transpose(
                        pkt[:, u * C : (u + 1) * C], kvv[:, cc, u, 0, :], ident
                    )
                ktb = h16.tile([128, U * C], F16, tag="ktb")
                nc.scalar.copy(ktb[:], pkt[:])

                # qdec = q * SCALE*lam^(i+1)  (gpsimd, independent of psum)
                qdec = h16.tile([128, U * C], F16, tag="qdec")
                nc.gpsimd.tensor_tensor(
                    qdec[:].rearrange("p (u i) -> p u i", u=U),
                    qbv[:, cc],
                    qdm_sb[:].rearrange("p (u i) -> p u i", u=U),
                    mybir.AluOpType.mult,
                )

                # kd[j, (u,d)] = k * lam_u^(127-j)
                kd = h16.tile([128, U * D], F16, tag="kd")
                nc.vector.tensor_tensor(
                    kd[:].rearrange("p (u d) -> p u d", u=U),
                    kvv[:, cc, :, 0, :],
                    ckm_sb[:].rearrange("p (u d) -> p u d", u=U),
                    mybir.AluOpType.mult,
                )

                # W = K^T Q, then mask
                pw = psum.tile([128, U * C], F32, tag="w")
                for u in range(U):
                    nc.tensor.matmul(
                        pw[:, u * C : (u + 1) * C],
                        lhsT=ktb[:, u * C : (u + 1) * C],
                        rhs=qbv[:, cc, u, :],
                        start=True,
                        stop=True,
                    )
                wm = h16.tile([128, U * C], F16, tag="wm")
                nc.vector.tensor_tensor(
                    wm[:], pw[:], mask_sb[:], mybir.AluOpType.mult
                )

                # O = Wm^T V + qdec^T S
                po = psum.tile([128, U * D], F32, tag="o")
                for u in range(U):
                    ds = slice(u * D, (u + 1) * D)
                    nc.tensor.matmul(
                        po[:, ds],
                        lhsT=wm[:, u * C : (u + 1) * C],
                        rhs=kvv[:, cc, u, 1, :],
                        start=True,
                        stop=False,
                    )
                    nc.tensor.matmul(
                        po[:, ds],
                        lhsT=qdec[:, u * C : (u + 1) * C],
                        rhs=s_cur[:, ds],
                        start=False,
                        stop=True,
                    )

                # state update: S <- lam^128 S + kd^T V  (diag-matmul decay)
                ps = psum.tile([128, U * D], F32, tag="s")
                for u in range(U):
                    ds = slice(u * D, (u + 1) * D)
                    nc.tensor.matmul(
                        ps[:, ds],
                        lhsT=sdg_sb[:, ds],
                        rhs=s_cur[:, ds],
                        start=True,
                        stop=False,
                    )
                    nc.tensor.matmul(
                        ps[:, ds],
                        lhsT=kd[:, ds],
                        rhs=kvv[:, cc, u, 1, :],
                        start=False,
                        stop=True,
                    )
                s_new = state.tile([128, U * D], F16, tag="ssb")
                nc.vector.tensor_copy(s_new[:], ps[:])
                s_cur = s_new

                half = (b % 2) * BC + cc
                nc.scalar.copy(
                    ob[:, half * U * D : (half + 1) * U * D], po[:]
                )

            if b % 2 == 1:
                nc.scalar.dma_start(o[b // 2], ob[:])

    nc.compile()
    return nc


_NC_CACHE = []


def _get_nc():
    if not _NC_CACHE:
        _NC_CACHE.append(_build_nc())
    return _NC_CACHE[0]


def _core_consts(core):
    lam = _lambdas()
    i_idx = np.arange(C).astype(np.float64)
    cstv = np.zeros((128, CST_W), np.float16)
    eye = np.eye(128, dtype=np.float64)
    for u in range(U):
        h = (U * core + u) % H
        l = lam[h]
        m = np.where(
            i_idx[None, :] >= i_idx[:, None],
            SCALE * l ** (i_idx[None, :] - i_idx[:, None]),
            0.0,
        )
        cstv[:, CST_MASK + u * C : CST_MASK + (u + 1) * C] = m.astype(np.float16)
        cq = (SCALE * l ** (i_idx + 1)).astype(np.float16)
        cstv[:, CST_QDM + u * C : CST_QDM + (u + 1) * C] = np.tile(cq, (128, 1))
        ck = (l ** (127.0 - i_idx)).astype(np.float16)
        cstv[:, CST_CKM + u * C : CST_CKM + (u + 1) * C] = np.repeat(
            ck[:, None], C, axis=1
        )
        cstv[:, CST_SDG + u * C : CST_SDG + (u + 1) * C] = (l**C * eye).astype(
            np.float16
        )
    cstv[:, CST_ID : CST_W] = np.eye(128, dtype=np.float16)
    return cstv


def kernel(query_states, key_states, value_states, initial_state):
    q16 = np.asarray(query_states).astype(np.float16)
    k16 = np.asarray(key_states).astype(np.float16)
    v16 = np.asarray(value_states).astype(np.float16)
    # [B,T,H,D] -> [B*H, T, D]
    q16 = np.transpose(q16, (0, 2, 1, 3)).reshape(B * H, T, D)
    k16 = np.transpose(k16, (0, 2, 1, 3)).reshape(B * H, T, D)
    v16 = np.transpose(v16, (0, 2, 1, 3)).reshape(B * H, T, D)
    s016 = np.asarray(initial_state).astype(np.float16).reshape(B * H, D, D)

    nc = _get_nc()
    in_maps = []
    for core in range(NCORES):
        lo = U * core
        cstv = _core_consts(core)
        qs = q16[lo : lo + U]  # [U, T, D]
        ks = k16[lo : lo + U]
        vs = v16[lo : lo + U]
        # qt[b, d, (cc,u,i)]
        qb = qs.reshape(U, NB, BC, C, D).transpose(1, 4, 2, 0, 3)
        qb = np.ascontiguousarray(qb.reshape(NB, 128, BC * U * C))
        # kv[b, p, (cc,u,x,d)]
        kvb = np.stack(
            [ks.reshape(U, NB, BC, C, D), vs.reshape(U, NB, BC, C, D)], axis=4
        )  # [U, NB, BC, C, 2, D]
        kvb = kvb.transpose(1, 3, 2, 0, 4, 5)  # [NB, C, BC, U, 2, D]
        kvb = np.ascontiguousarray(kvb.reshape(NB, 128, BC * U * 2 * D))
        s0b = np.ascontiguousarray(
            s016[lo : lo + U].transpose(1, 0, 2).reshape(128, U * D)
        )
        in_maps.append({"qt": qb, "kv": kvb, "s0": s0b, "cst": cstv})

    res = run_bass_kernel_spmd(
        nc, in_maps, core_ids=list(range(NCORES)), trace=TRACE
    )
    if TRACE:
        LAST["exec_time_ns"] = res.exec_time_ns
        LAST["mean_exec_time_ns"] = res.mean_exec_time_ns
        LAST["trace"] = (
            res.instructions_and_trace[1] if res.instructions_and_trace else None
        )

    out = np.empty((B * H, T, D), np.float32)
    for core in range(NCORES):
        ob = res.results[core]["o"].reshape(NB // 2, C, 2 * BC, U, D)
        # -> [U, NB//2, 2*BC, C, D] -> [U, T, D]
        out[U * core : U * core + U] = (
            ob.transpose(3, 0, 2, 1, 4).reshape(U, T, D).astype(np.float32)
        )
    return np.ascontiguousarray(
        np.transpose(out.reshape(B, H, T, D), (0, 2, 1, 3))
    )


# revision 31
# speedup vs baseline: 1.3647x; 1.0067x over previous
"""Chunked gated-linear-attention (GLA) kernel for Trainium2, 8 NeuronCores.

Math (per (b,h), per-head scalar decay lam):
    S_t = lam * S_{t-1} + k_t^T v_t ;  o_t = (q_t * SCALE) @ S_t

Block form, chunk C=128, state updated every chunk:
    chunk c:
      W[j,i]  = k_j.q_i * SCALE*lam^(i-j) * [j<=i]        (within-chunk)
      O[i]    = sum_j W[j,i] V[j] + SCALE*lam^(i+1) q_i . S
      S      <- lam^128 S + sum_j lam^(127-j) k_j v_j

Sharding: B*H = 32 (b,h) units, 4 per core (head-parallel, no collectives).
Host prep: cast fp16, pack Q pre-transposed [D,T] and K|V natural [T,2D]
into per-block (2-chunk) contiguous layouts so every DMA descriptor is
2-4KB per partition. K is loaded once; K^T for the W matmul is produced
on-chip via tensor-engine transposes. Output is written fp16 (host
upcasts to fp32). All matmuls fp16 -> PSUM fp32.
"""

import math
from contextlib import ExitStack

import numpy as np

import concourse.bacc as bacc
import concourse.mybir as mybir
import concourse.tile as tile
from concourse.bass_utils import run_bass_kernel_spmd

B, T, H, D = 2, 2048, 16, 128
C = 128                  # chunk size along time
NCH = T // C             # 16 chunks
BC = 2                   # chunks per DMA block
NB = NCH // BC           # 8 blocks
NCORES = 8
U = (B * H) // NCORES    # 4 (b,h) units per core
SCALE = 0.08838834764831845
LAYER_IDX, NUM_LAYERS = 12, 32

F32 = mybir.dt.float32
F16 = mybir.dt.float16

# cstA column offsets: [maskc | ckm (x BC) | sdg | ident]
CA_MASK = 0
CA_CKM = U * C
CA_SDG = CA_CKM + BC * U * C
CA_ID = CA_SDG + U * C
CA_W = CA_ID + C
# cstB: [qdm (x BC)]
CB_W = BC * U * C
NWARM = 30               # PE warm-up matmuls at body start

TRACE = False            # test.py sets True to capture an NTFF profile
LAST = {}


def _slopes(n):
    def p2(m):
        start = 2.0 ** (-(2.0 ** (-(math.log2(m) - 3))))
        return [start * start**i for i in range(m)]

    if math.log2(n).is_integer():
        return p2(n)
    cp = 2 ** math.floor(math.log2(n))
    return p2(cp) + _slopes(2 * cp)[0::2][: n - cp]


def _lambdas():
    s = -np.asarray(_slopes(H), dtype=np.float64) * (
        1.0 - LAYER_IDX / (NUM_LAYERS - 1) + 1e-5
    )
    return np.exp(s)


def _build_nc():
    nc = bacc.Bacc(trn_type="TRN2", debug=False, num_devices=NCORES)

    # qt[b, d, (cc, u, i)] : Q pre-transposed, per 2-chunk block
    qt = nc.dram_tensor("qt", [NB, 128, BC * U * C], F16, kind="ExternalInput")
    # kv[b, p, (cc, u, x, d)] : K|V natural layout, per 2-chunk block
    kv = nc.dram_tensor("kv", [NB, 128, BC * U * 2 * D], F16, kind="ExternalInput")
    # s0[dk, (u, dv)]
    s0 = nc.dram_tensor("s0", [128, U * D], F16, kind="ExternalInput")
    # cstA: maskc = SCALE*lam_u^(i-j) causal; ckm[j, (cc,u,d)] = lam_u^(127-j);
    # sdg[dk, (u,dk')] = lam_u^128 * I; ident.  cstB: qdm = SCALE*lam_u^(i+1)
    csta = nc.dram_tensor("csta", [128, CA_W], F16, kind="ExternalInput")
    cstb = nc.dram_tensor("cstb", [128, CB_W], F16, kind="ExternalInput")
    # o[b2, i, (cc4, u, dv)] fp16, 4 chunks per store block
    o = nc.dram_tensor("o", [NB // 2, 128, 2 * BC * U * D], F16, kind="ExternalOutput")

    with tile.TileContext(nc) as tc, ExitStack() as ctx:
        const = ctx.enter_context(tc.tile_pool(name="const", bufs=1))
        ld = ctx.enter_context(tc.tile_pool(name="ld", bufs=1))
        h16 = ctx.enter_context(tc.tile_pool(name="h16", bufs=3))
        outp = ctx.enter_context(tc.tile_pool(name="outp", bufs=3))
        state = ctx.enter_context(tc.tile_pool(name="state", bufs=2))
        psum = ctx.enter_context(tc.tile_pool(name="psum", bufs=2, space="PSUM"))

        # consts on the ACT (scalar) DMA ring, first-needed first; bulk
        # loads on the SP (sync) ring, kv before q within each block
        csta_sb = const.tile([128, CA_W], F16)
        nc.scalar.dma_start(csta_sb[:], csta[:])
        s_cur = state.tile([128, U * D], F16, tag="ssb")
        nc.scalar.dma_start(s_cur[:], s0[:])
        cstb_sb = const.tile([128, CB_W], F16)
        nc.scalar.dma_start(cstb_sb[:], cstb[:])

        mask_sb = csta_sb[:, CA_MASK:CA_CKM]
        ckm_sb = csta_sb[:, CA_CKM:CA_SDG]
        sdg_sb = csta_sb[:, CA_SDG:CA_ID]
        ident = csta_sb[:, CA_ID:CA_W]
        qdm_sb = cstb_sb[:]

        qtiles, kvtiles = [], []
        for b in range(NB):
            kvb = ld.tile(
                [128, BC * U * 2 * D], F16, tag="kvb", bufs=NB, name=f"kvb{b}"
            )
            nc.sync.dma_start(kvb[:], kv[b])
            qb = ld.tile([128, BC * U * C], F16, tag="qb", bufs=NB, name=f"qb{b}")
            nc.sync.dma_start(qb[:], qt[b])
            qtiles.append(qb)
            kvtiles.append(kvb)

        # PE warm-up: dependency-free matmuls on a memset tile keep the
        # HAM clock gate open while the first DMAs stream in
        zwu = const.tile([128, 128], F16)
        nc.vector.memset(zwu[:], 0.0)
        pwu = psum.tile([128, U * C], F32, tag="w", name="pwu")
        for _ in range(NWARM):
            nc.tensor.matmul(
                pwu[:, 0:128], lhsT=zwu[:], rhs=zwu[:], start=True, stop=True
            )

        def views(bn):
            qbv = qtiles[bn][:].rearrange("p (cc u i) -> p cc u i", cc=BC, u=U)
            kvv = kvtiles[bn][:].rearrange(
                "p (cc u x d) -> p cc u x d", cc=BC, u=U, x=2
            )
            return qbv, kvv

        blockres = {}

        def pre_transposes(bn, half):
            """4 K-chunk transposes for chunk `half` of block bn."""
            _, kvv = views(bn)
            if half == 0:
                pkt = psum.tile(
                    [128, BC * U * C], F16, tag="kt", name=f"pkt{bn}"
                )
                blockres[bn] = {"pkt": pkt}
            pkt = blockres[bn]["pkt"]
            for u in range(U):
                nc.tensor.transpose(
                    pkt[:, (half * U + u) * C : (half * U + u + 1) * C],
                    kvv[:, half, u, 0, :],
                    ident,
                )

        def pre_qdec(bn):
            qdec = h16.tile([128, BC * U * C], F16, tag="qdec", bufs=2)
            nc.gpsimd.tensor_tensor(
                qdec[:], qtiles[bn][:], qdm_sb, mybir.AluOpType.mult
            )
            blockres[bn]["qdec"] = qdec

        def pre_finish(bn):
            _, kvv = views(bn)
            ktb = h16.tile([128, BC * U * C], F16, tag="ktb", bufs=2)
            nc.vector.tensor_copy(ktb[:], blockres[bn]["pkt"][:])
            kd = h16.tile([128, BC * U * D], F16, tag="kd", bufs=2)
            nc.vector.tensor_tensor(
                kd[:].rearrange("p (cc u d) -> p cc u d", cc=BC, u=U),
                kvv[:, :, :, 0, :],
                ckm_sb.rearrange("p (cc u d) -> p cc u d", cc=BC, u=U),
                mybir.AluOpType.mult,
            )
            blockres[bn]["ktb"] = ktb
            blockres[bn]["kd"] = kd

        # full preamble for block 0
        pre_transposes(0, 0)
        pre_transposes(0, 1)
        pre_qdec(0)
        pre_finish(0)

        for b in range(NB):
            qbv, kvv = views(b)
            br = blockres[b]
            ktb, kd, qdec = br["ktb"], br["kd"], br["qdec"]
            if b % 2 == 0:
                ob = outp.tile([128, 2 * BC * U * D], F16, tag="ob")
            po = psum.tile([128, BC * U * D], F32, tag="o", bufs=1)

            for cc in range(BC):
                # W = K^T Q
                pw = psum.tile([128, U * C], F32, tag="w")
                for u in range(U):
                    nc.tensor.matmul(
                        pw[:, u * C : (u + 1) * C],
                        lhsT=ktb[:, (cc * U + u) * C : (cc * U + u + 1) * C],
                        rhs=qbv[:, cc, u, :],
                        start=True,
                        stop=True,
                    )

                # state update: S <- lam^128 S + kd^T V  (diag-matmul decay)
                ps = psum.tile([128, U * D], F32, tag="s")
                for u in range(U):
                    ds = slice(u * D, (u + 1) * D)
                    nc.tensor.matmul(
                        ps[:, ds],
                        lhsT=sdg_sb[:, ds],
                        rhs=s_cur[:, ds],
                        start=True,
                        stop=False,
                    )
                    nc.tensor.matmul(
                        ps[:, ds],
                        lhsT=kd[:, (cc * U + u) * D : (cc * U + u + 1) * D],
                        rhs=kvv[:, cc, u, 1, :],
                        start=False,
                        stop=True,
                    )

                # state cast first in the ACT queue (shortens the S chain)
                s_new = state.tile([128, U * D], F16, tag="ssb")
                nc.scalar.copy(s_new[:], ps[:])

                # mask W
                wm = h16.tile([128, U * C], F16, tag="wm")
                nc.vector.tensor_tensor(
                    wm[:], pw[:], mask_sb[:], mybir.AluOpType.mult
                )

                # O = Wm^T V + qdec^T S  (uses pre-update state)
                for u in range(U):
                    ds = slice((cc * U + u) * D, (cc * U + u + 1) * D)
                    nc.tensor.matmul(
                        po[:, ds],
                        lhsT=wm[:, u * C : (u + 1) * C],
                        rhs=kvv[:, cc, u, 1, :],
                        start=True,
                        stop=False,
                    )
                    nc.tensor.matmul(
                        po[:, ds],
                        lhsT=qdec[:, (cc * U + u) * C : (cc * U + u + 1) * C],
                        rhs=s_cur[:, ds.start - cc * U * D : ds.stop - cc * U * D],
                        start=False,
                        stop=True,
                    )
                s_cur = s_new

            # software-pipelined preamble for the next block (at block end,
            # so a late kv DMA never blocks ready work in the tensor queue)
            if b + 1 < NB:
                pre_transposes(b + 1, 0)
                pre_transposes(b + 1, 1)
                pre_qdec(b + 1)
                pre_finish(b + 1)

            nc.scalar.copy(
                ob[:, (b % 2) * BC * U * D : (b % 2 + 1) * BC * U * D], po[:]
            )
            del blockres[b]
            if b % 2 == 1:
                nc.scalar.dma_start(o[b // 2], ob[:])

    nc.compile()
    return nc


_NC_CACHE = []


def _get_nc():
    if not _NC_CACHE:
        _NC_CACHE.append(_build_nc())
    return _NC_CACHE[0]


def _core_consts(core):
    lam = _lambdas()
    i_idx = np.arange(C).astype(np.float64)
    cav = np.zeros((128, CA_W), np.float16)
    cbv = np.zeros((128, CB_W), np.float16)
    eye = np.eye(128, dtype=np.float64)
    for u in range(U):
        h = (U * core + u) % H
        l = lam[h]
        m = np.where(
            i_idx[None, :] >= i_idx[:, None],
            SCALE * l ** (i_idx[None, :] - i_idx[:, None]),
            0.0,
        )
        cav[:, CA_MASK + u * C : CA_MASK + (u + 1) * C] = m.astype(np.float16)
        cq = (SCALE * l ** (i_idx + 1)).astype(np.float16)
        ck = (l ** (127.0 - i_idx)).astype(np.float16)
        for cc in range(BC):
            off = (cc * U + u) * C
            cbv[:, off : off + C] = np.tile(cq, (128, 1))
            cav[:, CA_CKM + off : CA_CKM + off + C] = np.repeat(
                ck[:, None], C, axis=1
            )
        cav[:, CA_SDG + u * C : CA_SDG + (u + 1) * C] = (l**C * eye).astype(
            np.float16
        )
    cav[:, CA_ID : CA_W] = np.eye(128, dtype=np.float16)
    return cav, cbv


def kernel(query_states, key_states, value_states, initial_state):
    q16 = np.asarray(query_states).astype(np.float16)
    k16 = np.asarray(key_states).astype(np.float16)
    v16 = np.asarray(value_states).astype(np.float16)
    # [B,T,H,D] -> [B*H, T, D]
    q16 = np.transpose(q16, (0, 2, 1, 3)).reshape(B * H, T, D)
    k16 = np.transpose(k16, (0, 2, 1, 3)).reshape(B * H, T, D)
    v16 = np.transpose(v16, (0, 2, 1, 3)).reshape(B * H, T, D)
    s016 = np.asarray(initial_state).astype(np.float16).reshape(B * H, D, D)

    nc = _get_nc()
    in_maps = []
    for core in range(NCORES):
        lo = U * core
        cav, cbv = _core_consts(core)
        qs = q16[lo : lo + U]  # [U, T, D]
        ks = k16[lo : lo + U]
        vs = v16[lo : lo + U]
        # qt[b, d, (cc,u,i)]
        qb = qs.reshape(U, NB, BC, C, D).transpose(1, 4, 2, 0, 3)
        qb = np.ascontiguousarray(qb.reshape(NB, 128, BC * U * C))
        # kv[b, p, (cc,u,x,d)]
        kvb = np.stack(
            [ks.reshape(U, NB, BC, C, D), vs.reshape(U, NB, BC, C, D)], axis=4
        )  # [U, NB, BC, C, 2, D]
        kvb = kvb.transpose(1, 3, 2, 0, 4, 5)  # [NB, C, BC, U, 2, D]
        kvb = np.ascontiguousarray(kvb.reshape(NB, 128, BC * U * 2 * D))
        s0b = np.ascontiguousarray(
            s016[lo : lo + U].transpose(1, 0, 2).reshape(128, U * D)
        )
        in_maps.append(
            {"qt": qb, "kv": kvb, "s0": s0b, "csta": cav, "cstb": cbv}
        )

    res = run_bass_kernel_spmd(
        nc, in_maps, core_ids=list(range(NCORES)), trace=TRACE
    )
    if TRACE:
        LAST["exec_time_ns"] = res.exec_time_ns
        LAST["mean_exec_time_ns"] = res.mean_exec_time_ns
        LAST["trace"] = (
            res.instructions_and_trace[1] if res.instructions_and_trace else None
        )

    out = np.empty((B * H, T, D), np.float32)
    for core in range(NCORES):
        ob = res.results[core]["o"].reshape(NB // 2, C, 2 * BC, U, D)
        # -> [U, NB//2, 2*BC, C, D] -> [U, T, D]
        out[U * core : U * core + U] = (
            ob.transpose(3, 0, 2, 1, 4).reshape(U, T, D).astype(np.float32)
        )
    return np.ascontiguousarray(
        np.transpose(out.reshape(B, H, T, D), (0, 2, 1, 3))
    )


# revision 41
# speedup vs baseline: 1.3791x; 1.0105x over previous
"""Chunked gated-linear-attention (GLA) kernel for Trainium2, 8 NeuronCores.

Math (per (b,h), per-head scalar decay lam):
    S_t = lam * S_{t-1} + k_t^T v_t ;  o_t = (q_t * SCALE) @ S_t

Block form, chunk C=128, state updated every chunk:
    chunk c:
      W[j,i]  = k_j.q_i * SCALE*lam^(i-j) * [j<=i]        (within-chunk)
      O[i]    = sum_j W[j,i] V[j] + SCALE*lam^(i+1) q_i . S
      S      <- lam^128 S + sum_j lam^(127-j) k_j v_j

Sharding: B*H = 32 (b,h) units, 4 per core (head-parallel, no collectives).
Host prep: cast fp16, pack Q pre-transposed [D,T] and K|V natural [T,2D]
into per-block (2-chunk) contiguous layouts so every DMA descriptor is
2-4KB per partition. K is loaded once; K^T for the W matmul is produced
on-chip via tensor-engine transposes. Output is written fp16 (host
upcasts to fp32). All matmuls fp16 -> PSUM fp32.
"""

import math
from contextlib import ExitStack

import numpy as np

import concourse.bacc as bacc
import concourse.mybir as mybir
import concourse.tile as tile
from concourse.bass_utils import run_bass_kernel_spmd

B, T, H, D = 2, 2048, 16, 128
C = 128                  # chunk size along time
NCH = T // C             # 16 chunks
BC = 2                   # chunks per DMA block
NB = NCH // BC           # 8 blocks
NCORES = 8
U = (B * H) // NCORES    # 4 (b,h) units per core
SCALE = 0.08838834764831845
LAYER_IDX, NUM_LAYERS = 12, 32

F32 = mybir.dt.float32
F16 = mybir.dt.float16

# cst0: [ident | maskc]; cst1: [ckm (x BC) | sdg]; cstb: [qdm (x BC)]
C0_ID = 0
C0_MASK = C
C0_W = C + U * C
C1_CKM = 0
C1_SDG = BC * U * C
C1_W = BC * U * C + U * C
CB_W = BC * U * C
NWARM = 30               # PE warm-up matmuls at body start

TRACE = False            # test.py sets True to capture an NTFF profile
LAST = {}


def _slopes(n):
    def p2(m):
        start = 2.0 ** (-(2.0 ** (-(math.log2(m) - 3))))
        return [start * start**i for i in range(m)]

    if math.log2(n).is_integer():
        return p2(n)
    cp = 2 ** math.floor(math.log2(n))
    return p2(cp) + _slopes(2 * cp)[0::2][: n - cp]


def _lambdas():
    s = -np.asarray(_slopes(H), dtype=np.float64) * (
        1.0 - LAYER_IDX / (NUM_LAYERS - 1) + 1e-5
    )
    return np.exp(s)


def _build_nc():
    nc = bacc.Bacc(trn_type="TRN2", debug=False, num_devices=NCORES)

    # qt[b, d, (cc, u, i)] : Q pre-transposed, per 2-chunk block
    qt = nc.dram_tensor("qt", [NB, 128, BC * U * C], F16, kind="ExternalInput")
    # kv[b, p, (cc, u, x, d)] : K|V natural layout, per 2-chunk block
    kv = nc.dram_tensor("kv", [NB, 128, BC * U * 2 * D], F16, kind="ExternalInput")
    # s0[dk, (u, dv)]
    s0 = nc.dram_tensor("s0", [128, U * D], F16, kind="ExternalInput")
    # cst0: ident | maskc = SCALE*lam_u^(i-j) causal
    # cst1: ckm[j, (cc,u,d)] = lam_u^(127-j) | sdg[dk, (u,dk')] = lam_u^128 * I
    # cstb: qdm = SCALE*lam_u^(i+1) (bcast over partitions)
    cst0 = nc.dram_tensor("cst0", [128, C0_W], F16, kind="ExternalInput")
    cst1 = nc.dram_tensor("cst1", [128, C1_W], F16, kind="ExternalInput")
    cstb = nc.dram_tensor("cstb", [128, CB_W], F16, kind="ExternalInput")
    # o[b, i, (cc, u, dv)] fp16, one store block per 2-chunk block
    o = nc.dram_tensor("o", [NB, 128, BC * U * D], F16, kind="ExternalOutput")

    with tile.TileContext(nc) as tc, ExitStack() as ctx:
        const = ctx.enter_context(tc.tile_pool(name="const", bufs=1))
        ld = ctx.enter_context(tc.tile_pool(name="ld", bufs=1))
        h16 = ctx.enter_context(tc.tile_pool(name="h16", bufs=3))
        outp = ctx.enter_context(tc.tile_pool(name="outp", bufs=3))
        state = ctx.enter_context(tc.tile_pool(name="state", bufs=2))
        psum = ctx.enter_context(tc.tile_pool(name="psum", bufs=2, space="PSUM"))

        # ALL loads on the SP (sync) ring in strict first-need order; the
        # ACT (scalar) ring carries only output stores. Ring order: block-0
        # data, then consts as each pipeline stage first needs them.
        qtiles, kvtiles = [], []

        def load_block(bn):
            kvb = ld.tile(
                [128, BC * U * 2 * D], F16, tag="kvb", bufs=NB, name=f"kvb{bn}"
            )
            nc.sync.dma_start(kvb[:], kv[bn])
            qb = ld.tile(
                [128, BC * U * C], F16, tag="qb", bufs=NB, name=f"qb{bn}"
            )
            nc.sync.dma_start(qb[:], qt[bn])
            qtiles.append(qb)
            kvtiles.append(kvb)

        load_block(0)
        cst0_sb = const.tile([128, C0_W], F16)
        nc.sync.dma_start(cst0_sb[:], cst0[:])
        cst1_sb = const.tile([128, C1_W], F16)
        nc.sync.dma_start(cst1_sb[:], cst1[:])
        s_cur = state.tile([128, U * D], F16, tag="ssb")
        nc.sync.dma_start(s_cur[:], s0[:])
        cstb_sb = const.tile([128, CB_W], F16)
        nc.sync.dma_start(cstb_sb[:], cstb[:])
        for b in range(1, NB):
            load_block(b)

        ident = cst0_sb[:, C0_ID:C0_MASK]
        mask_sb = cst0_sb[:, C0_MASK:C0_W]
        ckm_sb = cst1_sb[:, C1_CKM:C1_SDG]
        sdg_sb = cst1_sb[:, C1_SDG:C1_W]
        qdm_sb = cstb_sb[:]

        # PE warm-up: dependency-free matmuls on a memset tile keep the
        # HAM clock gate open while the first DMAs stream in
        zwu = const.tile([128, 128], F16)
        nc.vector.memset(zwu[:], 0.0)
        pwu = psum.tile([128, U * C], F32, tag="w", name="pwu")
        for _ in range(NWARM):
            nc.tensor.matmul(
                pwu[:, 0:128], lhsT=zwu[:], rhs=zwu[:], start=True, stop=True
            )

        def views(bn):
            qbv = qtiles[bn][:].rearrange("p (cc u i) -> p cc u i", cc=BC, u=U)
            kvv = kvtiles[bn][:].rearrange(
                "p (cc u x d) -> p cc u x d", cc=BC, u=U, x=2
            )
            return qbv, kvv

        blockres = {}

        def pre_transposes(bn, half):
            """4 K-chunk transposes for chunk `half` of block bn."""
            _, kvv = views(bn)
            if half == 0:
                pkt = psum.tile(
                    [128, BC * U * C], F16, tag="kt", name=f"pkt{bn}"
                )
                blockres[bn] = {"pkt": pkt}
            pkt = blockres[bn]["pkt"]
            for u in range(U):
                nc.tensor.transpose(
                    pkt[:, (half * U + u) * C : (half * U + u + 1) * C],
                    kvv[:, half, u, 0, :],
                    ident,
                )

        def pre_qdec(bn):
            # block 0 on DVE (gpsimd's Q7 lib load lands late in the
            # prologue); steady-state blocks on gpsimd to offload DVE
            eng = nc.vector if bn == 0 else nc.gpsimd
            qdec = h16.tile([128, BC * U * C], F16, tag="qdec", bufs=2)
            eng.tensor_tensor(
                qdec[:], qtiles[bn][:], qdm_sb, mybir.AluOpType.mult
            )
            blockres[bn]["qdec"] = qdec

        def pre_finish(bn):
            _, kvv = views(bn)
            ktb = h16.tile([128, BC * U * C], F16, tag="ktb", bufs=2)
            nc.vector.tensor_copy(ktb[:], blockres[bn]["pkt"][:])
            kd = h16.tile([128, BC * U * D], F16, tag="kd", bufs=2)
            nc.vector.tensor_tensor(
                kd[:].rearrange("p (cc u d) -> p cc u d", cc=BC, u=U),
                kvv[:, :, :, 0, :],
                ckm_sb.rearrange("p (cc u d) -> p cc u d", cc=BC, u=U),
                mybir.AluOpType.mult,
            )
            blockres[bn]["ktb"] = ktb
            blockres[bn]["kd"] = kd

        # full preamble for block 0
        pre_transposes(0, 0)
        pre_transposes(0, 1)
        pre_qdec(0)
        pre_finish(0)

        for b in range(NB):
            qbv, kvv = views(b)
            br = blockres[b]
            ktb, kd, qdec = br["ktb"], br["kd"], br["qdec"]
            ob = outp.tile([128, BC * U * D], F16, tag="ob")
            po = psum.tile([128, BC * U * D], F32, tag="o", bufs=1)

            for cc in range(BC):
                # W = K^T Q
                pw = psum.tile([128, U * C], F32, tag="w")
                for u in range(U):
                    nc.tensor.matmul(
                        pw[:, u * C : (u + 1) * C],
                        lhsT=ktb[:, (cc * U + u) * C : (cc * U + u + 1) * C],
                        rhs=qbv[:, cc, u, :],
                        start=True,
                        stop=True,
                    )

                # state update: S <- lam^128 S + kd^T V  (diag-matmul decay)
                ps = psum.tile([128, U * D], F32, tag="s")
                for u in range(U):
                    ds = slice(u * D, (u + 1) * D)
                    nc.tensor.matmul(
                        ps[:, ds],
                        lhsT=sdg_sb[:, ds],
                        rhs=s_cur[:, ds],
                        start=True,
                        stop=False,
                    )
                    nc.tensor.matmul(
                        ps[:, ds],
                        lhsT=kd[:, (cc * U + u) * D : (cc * U + u + 1) * D],
                        rhs=kvv[:, cc, u, 1, :],
                        start=False,
                        stop=True,
                    )

                # state cast first in the ACT queue (shortens the S chain)
                s_new = state.tile([128, U * D], F16, tag="ssb")
                nc.scalar.copy(s_new[:], ps[:])

                # mask W
                wm = h16.tile([128, U * C], F16, tag="wm")
                nc.vector.tensor_tensor(
                    wm[:], pw[:], mask_sb[:], mybir.AluOpType.mult
                )

                # O = Wm^T V + qdec^T S  (uses pre-update state)
                for u in range(U):
                    ds = slice((cc * U + u) * D, (cc * U + u + 1) * D)
                    nc.tensor.matmul(
                        po[:, ds],
                        lhsT=wm[:, u * C : (u + 1) * C],
                        rhs=kvv[:, cc, u, 1, :],
                        start=True,
                        stop=False,
                    )
                    nc.tensor.matmul(
                        po[:, ds],
                        lhsT=qdec[:, (cc * U + u) * C : (cc * U + u + 1) * C],
                        rhs=s_cur[:, ds.start - cc * U * D : ds.stop - cc * U * D],
                        start=False,
                        stop=True,
                    )
                # per-chunk output copy so the last store isn't gated on a
                # full-block copy
                nc.scalar.copy(
                    ob[:, cc * U * D : (cc + 1) * U * D],
                    po[:, cc * U * D : (cc + 1) * U * D],
                )
                s_cur = s_new

            # software-pipelined preamble for the next block (at block end,
            # so a late kv DMA never blocks ready work in the tensor queue)
            if b + 1 < NB:
                pre_transposes(b + 1, 0)
                pre_transposes(b + 1, 1)
                pre_qdec(b + 1)
                pre_finish(b + 1)

            del blockres[b]
            nc.scalar.dma_start(o[b], ob[:])

    nc.compile()
    return nc


_NC_CACHE = []


def _get_nc():
    if not _NC_CACHE:
        _NC_CACHE.append(_build_nc())
    return _NC_CACHE[0]


def _core_consts(core):
    lam = _lambdas()
    i_idx = np.arange(C).astype(np.float64)
    c0v = np.zeros((128, C0_W), np.float16)
    c1v = np.zeros((128, C1_W), np.float16)
    cbv = np.zeros((128, CB_W), np.float16)
    eye = np.eye(128, dtype=np.float64)
    c0v[:, C0_ID : C0_ID + C] = np.eye(128, dtype=np.float16)
    for u in range(U):
        h = (U * core + u) % H
        l = lam[h]
        m = np.where(
            i_idx[None, :] >= i_idx[:, None],
            SCALE * l ** (i_idx[None, :] - i_idx[:, None]),
            0.0,
        )
        c0v[:, C0_MASK + u * C : C0_MASK + (u + 1) * C] = m.astype(np.float16)
        cq = (SCALE * l ** (i_idx + 1)).astype(np.float16)
        ck = (l ** (127.0 - i_idx)).astype(np.float16)
        for cc in range(BC):
            off = (cc * U + u) * C
            cbv[:, off : off + C] = np.tile(cq, (128, 1))
            c1v[:, C1_CKM + off : C1_CKM + off + C] = np.repeat(
                ck[:, None], C, axis=1
            )
        c1v[:, C1_SDG + u * C : C1_SDG + (u + 1) * C] = (l**C * eye).astype(
            np.float16
        )
    return c0v, c1v, cbv


def kernel(query_states, key_states, value_states, initial_state):
    q16 = np.asarray(query_states).astype(np.float16)
    k16 = np.asarray(key_states).astype(np.float16)
    v16 = np.asarray(value_states).astype(np.float16)
    # [B,T,H,D] -> [B*H, T, D]
    q16 = np.transpose(q16, (0, 2, 1, 3)).reshape(B * H, T, D)
    k16 = np.transpose(k16, (0, 2, 1, 3)).reshape(B * H, T, D)
    v16 = np.transpose(v16, (0, 2, 1, 3)).reshape(B * H, T, D)
    s016 = np.asarray(initial_state).astype(np.float16).reshape(B * H, D, D)

    nc = _get_nc()
    in_maps = []
    for core in range(NCORES):
        lo = U * core
        c0v, c1v, cbv = _core_consts(core)
        qs = q16[lo : lo + U]  # [U, T, D]
        ks = k16[lo : lo + U]
        vs = v16[lo : lo + U]
        # qt[b, d, (cc,u,i)]
        qb = qs.reshape(U, NB, BC, C, D).transpose(1, 4, 2, 0, 3)
        qb = np.ascontiguousarray(qb.reshape(NB, 128, BC * U * C))
        # kv[b, p, (cc,u,x,d)]
        kvb = np.stack(
            [ks.reshape(U, NB, BC, C, D), vs.reshape(U, NB, BC, C, D)], axis=4
        )  # [U, NB, BC, C, 2, D]
        kvb = kvb.transpose(1, 3, 2, 0, 4, 5)  # [NB, C, BC, U, 2, D]
        kvb = np.ascontiguousarray(kvb.reshape(NB, 128, BC * U * 2 * D))
        s0b = np.ascontiguousarray(
            s016[lo : lo + U].transpose(1, 0, 2).reshape(128, U * D)
        )
        in_maps.append(
            {
                "qt": qb,
                "kv": kvb,
                "s0": s0b,
                "cst0": c0v,
                "cst1": c1v,
                "cstb": cbv,
            }
        )

    res = run_bass_kernel_spmd(
        nc, in_maps, core_ids=list(range(NCORES)), trace=TRACE
    )
    if TRACE:
        LAST["exec_time_ns"] = res.exec_time_ns
        LAST["mean_exec_time_ns"] = res.mean_exec_time_ns
        LAST["trace"] = (
            res.instructions_and_trace[1] if res.instructions_and_trace else None
        )

    out = np.empty((B * H, T, D), np.float32)
    for core in range(NCORES):
        ob = res.results[core]["o"].reshape(NB, C, BC, U, D)
        # -> [U, NB, BC, C, D] -> [U, T, D]
        out[U * core : U * core + U] = (
            ob.transpose(3, 0, 2, 1, 4).reshape(U, T, D).astype(np.float32)
        )
    return np.ascontiguousarray(
        np.transpose(out.reshape(B, H, T, D), (0, 2, 1, 3))
    )


# revision 54
# speedup vs baseline: 1.4720x; 1.0674x over previous
"""Chunked gated-linear-attention (GLA) kernel for Trainium2, 8 NeuronCores.

Math (per (b,h), per-head scalar decay lam):
    S_t = lam * S_{t-1} + k_t^T v_t ;  o_t = (q_t * SCALE) @ S_t

Block form, chunk C=128, state updated every chunk. Host pre-scales
    qs_i = q_i * SCALE * lam^(i-64)        (i = index within chunk)
so on-chip:
    W[j,i]  = (k_j . qs_i) * lam^(64-j) * [j<=i]     (mask is causal*lam^(64-j))
    O[i]    = sum_j W[j,i] V[j] + qs_i . S'          (S' = lam^65 * S)
    S'     <- lam^128 S' + sum_j lam^(192-j) k_j v_j

Sharding: B*H = 32 (b,h) units, 4 per core (head-parallel, no collectives).
DMA: all inputs packed per 2-chunk block as [128, X] with 2-4KB/partition
contiguous descriptors, streamed on the sync (SP) HWDGE ring in exact
first-need order; output fp16 stores per block on the scalar (ACT) ring.
K is loaded once; K^T comes from tensor-engine transposes. The O-stage
(WV + qS matmuls, output copies) is software-pipelined one chunk behind
the W/state stage so DVE mask/cast latency never stalls the tensor queue.
PE warm-up matmuls on a memset tile hold the HAM clock gate open during
the initial DMA fill.
"""

import math
from contextlib import ExitStack

import numpy as np

import concourse.bacc as bacc
import concourse.mybir as mybir
import concourse.tile as tile
from concourse.bass_utils import run_bass_kernel_spmd

B, T, H, D = 2, 2048, 16, 128
C = 128                  # chunk size along time
NCH = T // C             # 16 chunks
BC = 2                   # chunks per DMA block
NB = NCH // BC           # 8 blocks
NCORES = 8
U = (B * H) // NCORES    # 4 (b,h) units per core
SCALE = 0.08838834764831845
LAYER_IDX, NUM_LAYERS = 12, 32

F32 = mybir.dt.float32
F16 = mybir.dt.float16

# cst0: [ident | maskc]; cst1: [ckm (x BC) | sdg]; cstb: [qdm (x BC)]
C0_ID = 0
C0_MASK = C
C0_W = C + U * C
C1_CKM = 0
C1_SDG = BC * U * C
C1_W = BC * U * C + U * C
CB_W = BC * U * C
NWARM = 48               # PE warm-up matmuls at body start

TRACE = False            # test.py sets True to capture an NTFF profile
LAST = {}


def _slopes(n):
    def p2(m):
        start = 2.0 ** (-(2.0 ** (-(math.log2(m) - 3))))
        return [start * start**i for i in range(m)]

    if math.log2(n).is_integer():
        return p2(n)
    cp = 2 ** math.floor(math.log2(n))
    return p2(cp) + _slopes(2 * cp)[0::2][: n - cp]


def _lambdas():
    s = -np.asarray(_slopes(H), dtype=np.float64) * (
        1.0 - LAYER_IDX / (NUM_LAYERS - 1) + 1e-5
    )
    return np.exp(s)


def _build_nc():
    nc = bacc.Bacc(trn_type="TRN2", debug=False, num_devices=NCORES)

    # qt[b, d, (cc, u, i)] : Q^T, per 2-chunk block
    qt = nc.dram_tensor("qt", [NB, 128, BC * U * C], F16, kind="ExternalInput")
    # kv[b, p, (cc, u, x, d)] : K|V natural layout, per 2-chunk block
    kv = nc.dram_tensor("kv", [NB, 128, BC * U * 2 * D], F16, kind="ExternalInput")
    # s0[dk, (u, dv)]
    s0 = nc.dram_tensor("s0", [128, U * D], F16, kind="ExternalInput")
    cst0 = nc.dram_tensor("cst0", [128, C0_W], F16, kind="ExternalInput")
    cst1 = nc.dram_tensor("cst1", [128, C1_W], F16, kind="ExternalInput")
    cstb = nc.dram_tensor("cstb", [128, CB_W], F16, kind="ExternalInput")
    # o[b, i, (cc, u, dv)] fp16
    o = nc.dram_tensor("o", [NB, 128, BC * U * D], F16, kind="ExternalOutput")

    with tile.TileContext(nc) as tc, ExitStack() as ctx:
        const = ctx.enter_context(tc.tile_pool(name="const", bufs=1))
        ld = ctx.enter_context(tc.tile_pool(name="ld", bufs=1))
        h16 = ctx.enter_context(tc.tile_pool(name="h16", bufs=3))
        outp = ctx.enter_context(tc.tile_pool(name="outp", bufs=3))
        state = ctx.enter_context(tc.tile_pool(name="state", bufs=4))
        psum = ctx.enter_context(tc.tile_pool(name="psum", bufs=2, space="PSUM"))

        # ALL loads on the SP (sync) ring in strict first-need order; the
        # ACT (scalar) ring carries only output stores.
        qtiles, kvtiles = [], []

        def load_block(bn):
            kvb = ld.tile(
                [128, BC * U * 2 * D], F16, tag="kvb", bufs=NB, name=f"kvb{bn}"
            )
            nc.sync.dma_start(kvb[:], kv[bn])
            qb = ld.tile(
                [128, BC * U * C], F16, tag="qb", bufs=NB, name=f"qb{bn}"
            )
            nc.sync.dma_start(qb[:], qt[bn])
            qtiles.append(qb)
            kvtiles.append(kvb)

        load_block(0)
        cst0_sb = const.tile([128, C0_W], F16)
        nc.sync.dma_start(cst0_sb[:], cst0[:])
        cst1_sb = const.tile([128, C1_W], F16)
        nc.sync.dma_start(cst1_sb[:], cst1[:])
        s_cur = state.tile([128, U * D], F16, tag="ssb")
        nc.sync.dma_start(s_cur[:], s0[:])
        cstb_sb = const.tile([128, CB_W], F16)
        nc.sync.dma_start(cstb_sb[:], cstb[:])
        for b in range(1, NB):
            load_block(b)

        ident = cst0_sb[:, C0_ID:C0_MASK]
        mask_sb = cst0_sb[:, C0_MASK:C0_W]
        ckm_sb = cst1_sb[:, C1_CKM:C1_SDG]
        sdg_sb = cst1_sb[:, C1_SDG:C1_W]
        qdm_sb = cstb_sb[:]

        # PE warm-up: dependency-free matmuls on a memset tile keep the
        # HAM clock gate open while the first DMAs stream in
        zwu = const.tile([128, 128], F16)
        nc.vector.memset(zwu[:], 0.0)
        pwu = psum.tile([128, U * C], F32, tag="w", name="pwu")
        for _ in range(NWARM):
            nc.tensor.matmul(
                pwu[:, 0:128], lhsT=zwu[:], rhs=zwu[:], start=True, stop=True
            )

        def views(bn):
            qbv = qtiles[bn][:].rearrange("p (cc u i) -> p cc u i", cc=BC, u=U)
            kvv = kvtiles[bn][:].rearrange(
                "p (cc u x d) -> p cc u x d", cc=BC, u=U, x=2
            )
            return qbv, kvv

        blockres = {}
        obtiles = {}

        def pre_transposes(bn, half):
            _, kvv = views(bn)
            if half == 0:
                pkt = psum.tile(
                    [128, BC * U * C], F16, tag="kt", name=f"pkt{bn}"
                )
                blockres[bn] = {"pkt": pkt}
            pkt = blockres[bn]["pkt"]
            for u in range(U):
                nc.tensor.transpose(
                    pkt[:, (half * U + u) * C : (half * U + u + 1) * C],
                    kvv[:, half, u, 0, :],
                    ident,
                )

        def pre_finish(bn):
            _, kvv = views(bn)
            ktb = h16.tile([128, BC * U * C], F16, tag="ktb", bufs=2)
            nc.vector.tensor_copy(ktb[:], blockres[bn]["pkt"][:])
            kd = h16.tile([128, BC * U * D], F16, tag="kd", bufs=2)
            nc.vector.tensor_tensor(
                kd[:].rearrange("p (cc u d) -> p cc u d", cc=BC, u=U),
                kvv[:, :, :, 0, :],
                ckm_sb.rearrange("p (cc u d) -> p cc u d", cc=BC, u=U),
                mybir.AluOpType.mult,
            )
            # block 0 on DVE (gpsimd's Q7 lib load lands late in the
            # prologue); steady-state blocks on gpsimd to offload DVE
            eng = nc.vector if bn == 0 else nc.gpsimd
            qdec = h16.tile([128, BC * U * C], F16, tag="qdec", bufs=2)
            eng.tensor_tensor(
                qdec[:], qtiles[bn][:], qdm_sb, mybir.AluOpType.mult
            )
            blockres[bn]["ktb"] = ktb
            blockres[bn]["kd"] = kd
            blockres[bn]["qdec"] = qdec

        potiles = {}

        def emit_ostage(pv):
            """O = Wm^T V + qdec^T S for a pending chunk; copy+store per block."""
            pb, pcc, pwm, psv = pv
            _, pkvv = views(pb)
            pqdec = blockres[pb]["qdec"]
            if pcc == 0:
                potiles[pb] = psum.tile(
                    [128, BC * U * D], F32, tag="o", bufs=1, name=f"po{pb}"
                )
            po = potiles[pb]
            for u in range(U):
                ds = slice((pcc * U + u) * D, (pcc * U + u + 1) * D)
                ss = slice(u * D, (u + 1) * D)
                nc.tensor.matmul(
                    po[:, ds],
                    lhsT=pwm[:, u * C : (u + 1) * C],
                    rhs=pkvv[:, pcc, u, 1, :],
                    start=True,
                    stop=False,
                )
                nc.tensor.matmul(
                    po[:, ds],
                    lhsT=pqdec[:, (pcc * U + u) * C : (pcc * U + u + 1) * C],
                    rhs=psv[:, ss],
                    start=False,
                    stop=True,
                )
            if pcc == BC - 1:
                ob = outp.tile(
                    [128, BC * U * D], F16, tag="ob", name=f"ob{pb}"
                )
                nc.scalar.copy(ob[:], po[:])
                nc.scalar.dma_start(o[pb], ob[:])
                del potiles[pb]

        # full preamble for block 0
        pre_transposes(0, 0)
        pre_transposes(0, 1)
        pre_finish(0)

        pending = None
        for b in range(NB):
            qbv, kvv = views(b)
            br = blockres[b]
            ktb, kd = br["ktb"], br["kd"]

            for cc in range(BC):
                # W = K^T Qs
                pw = psum.tile([128, U * C], F32, tag="w")
                for u in range(U):
                    nc.tensor.matmul(
                        pw[:, u * C : (u + 1) * C],
                        lhsT=ktb[:, (cc * U + u) * C : (cc * U + u + 1) * C],
                        rhs=qbv[:, cc, u, :],
                        start=True,
                        stop=True,
                    )

                # state update: S' <- lam^128 S' + kd^T V
                ps = psum.tile([128, U * D], F32, tag="s")
                for u in range(U):
                    ds = slice(u * D, (u + 1) * D)
                    nc.tensor.matmul(
                        ps[:, ds],
                        lhsT=sdg_sb[:, ds],
                        rhs=s_cur[:, ds],
                        start=True,
                        stop=False,
                    )
                    nc.tensor.matmul(
                        ps[:, ds],
                        lhsT=kd[:, (cc * U + u) * D : (cc * U + u + 1) * D],
                        rhs=kvv[:, cc, u, 1, :],
                        start=False,
                        stop=True,
                    )

                # state cast on ACT, ahead of output copies in its queue
                s_new = state.tile([128, U * D], F16, tag="ssb")
                nc.scalar.copy(s_new[:], ps[:])

                # mask W (causal * lam^(64-j))
                wm = h16.tile([128, U * C], F16, tag="wm")
                nc.vector.tensor_tensor(
                    wm[:], pw[:], mask_sb[:], mybir.AluOpType.mult
                )

                # O-stage of the PREVIOUS chunk (one-chunk software pipeline)
                if pending is not None:
                    emit_ostage(pending)

                # next-block preamble at block end
                if cc == BC - 1 and b + 1 < NB:
                    pre_transposes(b + 1, 0)
                    pre_transposes(b + 1, 1)
                    pre_finish(b + 1)

                pending = (b, cc, wm, s_cur)
                s_cur = s_new

            if b - 1 in blockres:
                del blockres[b - 1]

        emit_ostage(pending)

    nc.compile()
    return nc


_NC_CACHE = []


def _get_nc():
    if not _NC_CACHE:
        _NC_CACHE.append(_build_nc())
    return _NC_CACHE[0]


def _core_consts(core):
    lam = _lambdas()
    i_idx = np.arange(C).astype(np.float64)
    c0v = np.zeros((128, C0_W), np.float16)
    c1v = np.zeros((128, C1_W), np.float16)
    cbv = np.zeros((128, CB_W), np.float16)
    eye = np.eye(128, dtype=np.float64)
    c0v[:, C0_ID : C0_ID + C] = np.eye(128, dtype=np.float16)
    for u in range(U):
        h = (U * core + u) % H
        l = lam[h]
        m = np.where(
            i_idx[None, :] >= i_idx[:, None],
            SCALE * l ** (i_idx[None, :] - i_idx[:, None]),
            0.0,
        )
        c0v[:, C0_MASK + u * C : C0_MASK + (u + 1) * C] = m.astype(np.float16)
        cq = (SCALE * l ** (i_idx + 1)).astype(np.float16)
        ck = (l ** (127.0 - i_idx)).astype(np.float16)
        for cc in range(BC):
            off = (cc * U + u) * C
            cbv[:, off : off + C] = np.tile(cq, (128, 1))
            c1v[:, C1_CKM + off : C1_CKM + off + C] = np.repeat(
                ck[:, None], C, axis=1
            )
        c1v[:, C1_SDG + u * C : C1_SDG + (u + 1) * C] = (l**C * eye).astype(
            np.float16
        )
    return c0v, c1v, cbv


def kernel(query_states, key_states, value_states, initial_state):
    q16 = np.asarray(query_states).astype(np.float16)
    k16 = np.asarray(key_states).astype(np.float16)
    v16 = np.asarray(value_states).astype(np.float16)
    # [B,T,H,D] -> [B*H, T, D]
    q16 = np.transpose(q16, (0, 2, 1, 3)).reshape(B * H, T, D)
    k16 = np.transpose(k16, (0, 2, 1, 3)).reshape(B * H, T, D)
    v16 = np.transpose(v16, (0, 2, 1, 3)).reshape(B * H, T, D)
    s016 = np.asarray(initial_state).astype(np.float16).reshape(B * H, D, D)

    nc = _get_nc()
    in_maps = []
    for core in range(NCORES):
        lo = U * core
        c0v, c1v, cbv = _core_consts(core)
        qs = q16[lo : lo + U]  # [U, T, D]
        ks = k16[lo : lo + U]
        vs = v16[lo : lo + U]
        # qt[b, d, (cc,u,i)]
        qb = qs.reshape(U, NB, BC, C, D).transpose(1, 4, 2, 0, 3)
        qb = np.ascontiguousarray(qb.reshape(NB, 128, BC * U * C))
        # kv[b, p, (cc,u,x,d)]
        kvb = np.stack(
            [ks.reshape(U, NB, BC, C, D), vs.reshape(U, NB, BC, C, D)], axis=4
        )  # [U, NB, BC, C, 2, D]
        kvb = kvb.transpose(1, 3, 2, 0, 4, 5)  # [NB, C, BC, U, 2, D]
        kvb = np.ascontiguousarray(kvb.reshape(NB, 128, BC * U * 2 * D))
        s0b = np.ascontiguousarray(
            s016[lo : lo + U].transpose(1, 0, 2).reshape(128, U * D)
        )
        in_maps.append(
            {
                "qt": qb,
                "kv": kvb,
                "s0": s0b,
                "cst0": c0v,
                "cst1": c1v,
                "cstb": cbv,
            }
        )

    res = run_bass_kernel_spmd(
        nc, in_maps, core_ids=list(range(NCORES)), trace=TRACE
    )
    if TRACE:
        LAST["exec_time_ns"] = res.exec_time_ns
        LAST["mean_exec_time_ns"] = res.mean_exec_time_ns
        LAST["trace"] = (
            res.instructions_and_trace[1] if res.instructions_and_trace else None
        )

    out = np.empty((B * H, T, D), np.float32)
    for core in range(NCORES):
        ob = res.results[core]["o"].reshape(NB, C, BC, U, D)
        # -> [U, NB, BC, C, D] -> [U, T, D]
        out[U * core : U * core + U] = (
            ob.transpose(3, 0, 2, 1, 4).reshape(U, T, D).astype(np.float32)
        )
    return np.ascontiguousarray(
        np.transpose(out.reshape(B, H, T, D), (0, 2, 1, 3))
    )


# revision 56
# speedup vs baseline: 1.5136x; 1.0283x over previous
"""Chunked gated-linear-attention (GLA) kernel for Trainium2, 8 NeuronCores.

Math (per (b,h), per-head scalar decay lam):
    S_t = lam * S_{t-1} + k_t^T v_t ;  o_t = (q_t * SCALE) @ S_t

Block form, chunk C=128, state updated every chunk. Host pre-scales
    qs_i = q_i * SCALE * lam^(i-64)        (i = index within chunk)
so on-chip:
    W[j,i]  = (k_j . qs_i) * lam^(64-j) * [j<=i]     (mask is causal*lam^(64-j))
    O[i]    = sum_j W[j,i] V[j] + qs_i . S'          (S' = lam^65 * S)
    S'     <- lam^128 S' + sum_j lam^(192-j) k_j v_j

Sharding: B*H = 32 (b,h) units, 4 per core (head-parallel, no collectives).
DMA: all inputs packed per 2-chunk block as [128, X] with 2-4KB/partition
contiguous descriptors, streamed on the sync (SP) HWDGE ring in exact
first-need order; output fp16 stores per block on the scalar (ACT) ring.
K is loaded once; K^T comes from tensor-engine transposes. The O-stage
(WV + qS matmuls, output copies) is software-pipelined one chunk behind
the W/state stage so DVE mask/cast latency never stalls the tensor queue.
PE warm-up matmuls on a memset tile hold the HAM clock gate open during
the initial DMA fill.
"""

import math
from contextlib import ExitStack

import numpy as np

import concourse.bacc as bacc
import concourse.mybir as mybir
import concourse.tile as tile
from concourse.bass_utils import run_bass_kernel_spmd

B, T, H, D = 2, 2048, 16, 128
C = 128                  # chunk size along time
NCH = T // C             # 16 chunks
BC = 2                   # chunks per DMA block
NB = NCH // BC           # 8 blocks
NCORES = 8
U = (B * H) // NCORES    # 4 (b,h) units per core
SCALE = 0.08838834764831845
LAYER_IDX, NUM_LAYERS = 12, 32

F32 = mybir.dt.float32
F16 = mybir.dt.float16

# cst0: [ident | maskc]; cst1: [ckm (x BC) | sdg]; cstb: [qdm (x BC)]
C0_ID = 0
C0_MASK = C
C0_W = C + U * C
C1_CKM = 0
C1_SDG = BC * U * C
C1_W = BC * U * C + U * C
CB_W = BC * U * C
NWARM = 48               # PE warm-up matmuls at body start

TRACE = False            # test.py sets True to capture an NTFF profile
LAST = {}


def _slopes(n):
    def p2(m):
        start = 2.0 ** (-(2.0 ** (-(math.log2(m) - 3))))
        return [start * start**i for i in range(m)]

    if math.log2(n).is_integer():
        return p2(n)
    cp = 2 ** math.floor(math.log2(n))
    return p2(cp) + _slopes(2 * cp)[0::2][: n - cp]


def _lambdas():
    s = -np.asarray(_slopes(H), dtype=np.float64) * (
        1.0 - LAYER_IDX / (NUM_LAYERS - 1) + 1e-5
    )
    return np.exp(s)


def _build_nc():
    nc = bacc.Bacc(trn_type="TRN2", debug=False, num_devices=NCORES)

    # qt[b, d, (cc, u, i)] : Q^T, per 2-chunk block
    qt = nc.dram_tensor("qt", [NB, 128, BC * U * C], F16, kind="ExternalInput")
    # kv[b, p, (cc, u, x, d)] : K|V natural layout, per 2-chunk block
    kv = nc.dram_tensor("kv", [NB, 128, BC * U * 2 * D], F16, kind="ExternalInput")
    # s0[dk, (u, dv)]
    s0 = nc.dram_tensor("s0", [128, U * D], F16, kind="ExternalInput")
    cst0 = nc.dram_tensor("cst0", [128, C0_W], F16, kind="ExternalInput")
    cst1 = nc.dram_tensor("cst1", [128, C1_W], F16, kind="ExternalInput")
    cstb = nc.dram_tensor("cstb", [128, CB_W], F16, kind="ExternalInput")
    # o[b, i, (cc, u, dv)] fp16
    o = nc.dram_tensor("o", [NB, 128, BC * U * D], F16, kind="ExternalOutput")

    with tile.TileContext(nc) as tc, ExitStack() as ctx:
        const = ctx.enter_context(tc.tile_pool(name="const", bufs=1))
        ld = ctx.enter_context(tc.tile_pool(name="ld", bufs=1))
        h16 = ctx.enter_context(tc.tile_pool(name="h16", bufs=3))
        outp = ctx.enter_context(tc.tile_pool(name="outp", bufs=3))
        state = ctx.enter_context(tc.tile_pool(name="state", bufs=4))
        psum = ctx.enter_context(tc.tile_pool(name="psum", bufs=2, space="PSUM"))

        # ALL loads on the SP (sync) ring in strict first-need order; the
        # ACT (scalar) ring carries only output stores.
        qtiles, kvtiles = [], []

        def load_block(bn):
            kvb = ld.tile(
                [128, BC * U * 2 * D], F16, tag="kvb", bufs=NB, name=f"kvb{bn}"
            )
            nc.sync.dma_start(kvb[:], kv[bn])
            qb = ld.tile(
                [128, BC * U * C], F16, tag="qb", bufs=NB, name=f"qb{bn}"
            )
            nc.sync.dma_start(qb[:], qt[bn])
            qtiles.append(qb)
            kvtiles.append(kvb)

        load_block(0)
        cst0_sb = const.tile([128, C0_W], F16)
        nc.sync.dma_start(cst0_sb[:], cst0[:])
        cst1_sb = const.tile([128, C1_W], F16)
        nc.sync.dma_start(cst1_sb[:], cst1[:])
        s_cur = state.tile([128, U * D], F16, tag="ssb")
        nc.sync.dma_start(s_cur[:], s0[:])
        cstb_sb = const.tile([128, CB_W], F16)
        nc.sync.dma_start(cstb_sb[:], cstb[:])
        for b in range(1, NB):
            load_block(b)

        ident = cst0_sb[:, C0_ID:C0_MASK]
        mask_sb = cst0_sb[:, C0_MASK:C0_W]
        ckm_sb = cst1_sb[:, C1_CKM:C1_SDG]
        sdg_sb = cst1_sb[:, C1_SDG:C1_W]
        qdm_sb = cstb_sb[:]

        # PE warm-up: dependency-free matmuls on a memset tile keep the
        # HAM clock gate open while the first DMAs stream in
        zwu = const.tile([128, 128], F16)
        nc.vector.memset(zwu[:], 0.0)
        pwu = psum.tile([128, U * C], F32, tag="w", name="pwu")
        for _ in range(NWARM):
            nc.tensor.matmul(
                pwu[:, 0:128], lhsT=zwu[:], rhs=zwu[:], start=True, stop=True
            )

        def views(bn):
            qbv = qtiles[bn][:].rearrange("p (cc u i) -> p cc u i", cc=BC, u=U)
            kvv = kvtiles[bn][:].rearrange(
                "p (cc u x d) -> p cc u x d", cc=BC, u=U, x=2
            )
            return qbv, kvv

        blockres = {}
        obtiles = {}

        def pre_transposes(bn, half):
            _, kvv = views(bn)
            if half == 0:
                pkt = psum.tile(
                    [128, BC * U * C], F16, tag="kt", name=f"pkt{bn}"
                )
                blockres[bn] = {"pkt": pkt}
            pkt = blockres[bn]["pkt"]
            for u in range(U):
                nc.tensor.transpose(
                    pkt[:, (half * U + u) * C : (half * U + u + 1) * C],
                    kvv[:, half, u, 0, :],
                    ident,
                )

        def pre_finish(bn):
            _, kvv = views(bn)
            ktb = h16.tile([128, BC * U * C], F16, tag="ktb", bufs=2)
            nc.vector.tensor_copy(ktb[:], blockres[bn]["pkt"][:])
            kd = h16.tile([128, BC * U * D], F16, tag="kd", bufs=2)
            nc.vector.tensor_tensor(
                kd[:].rearrange("p (cc u d) -> p cc u d", cc=BC, u=U),
                kvv[:, :, :, 0, :],
                ckm_sb.rearrange("p (cc u d) -> p cc u d", cc=BC, u=U),
                mybir.AluOpType.mult,
            )
            # block 0 on DVE (gpsimd's Q7 lib load lands late in the
            # prologue); steady-state blocks on gpsimd to offload DVE
            eng = nc.vector if bn == 0 else nc.gpsimd
            qdec = h16.tile([128, BC * U * C], F16, tag="qdec", bufs=2)
            eng.tensor_tensor(
                qdec[:], qtiles[bn][:], qdm_sb, mybir.AluOpType.mult
            )
            blockres[bn]["ktb"] = ktb
            blockres[bn]["kd"] = kd
            blockres[bn]["qdec"] = qdec

        potiles = {}

        def emit_ostage(pv):
            """O = Wm^T V + qdec^T S for a pending chunk; copy+store per block."""
            pb, pcc, pwm, psv = pv
            _, pkvv = views(pb)
            pqdec = blockres[pb]["qdec"]
            if pcc == 0:
                potiles[pb] = psum.tile(
                    [128, BC * U * D], F32, tag="o", bufs=1, name=f"po{pb}"
                )
            po = potiles[pb]
            for u in range(U):
                ds = slice((pcc * U + u) * D, (pcc * U + u + 1) * D)
                ss = slice(u * D, (u + 1) * D)
                nc.tensor.matmul(
                    po[:, ds],
                    lhsT=pwm[:, u * C : (u + 1) * C],
                    rhs=pkvv[:, pcc, u, 1, :],
                    start=True,
                    stop=False,
                )
                nc.tensor.matmul(
                    po[:, ds],
                    lhsT=pqdec[:, (pcc * U + u) * C : (pcc * U + u + 1) * C],
                    rhs=psv[:, ss],
                    start=False,
                    stop=True,
                )
            if pb == NB - 1:
                # last block: per-chunk half copies + half stores so the
                # final store isn't gated on a full-block copy
                ob = blockres[pb].setdefault(
                    "ob",
                    outp.tile([128, BC * U * D], F16, tag="ob", name=f"ob{pb}"),
                )
                hs = slice(pcc * U * D, (pcc + 1) * U * D)
                nc.scalar.copy(ob[:, hs], po[:, hs])
                nc.scalar.dma_start(o[pb, :, hs], ob[:, hs])
                if pcc == BC - 1:
                    del potiles[pb]
            elif pcc == BC - 1:
                ob = outp.tile(
                    [128, BC * U * D], F16, tag="ob", name=f"ob{pb}"
                )
                nc.scalar.copy(ob[:], po[:])
                nc.scalar.dma_start(o[pb], ob[:])
                del potiles[pb]

        # full preamble for block 0
        pre_transposes(0, 0)
        pre_transposes(0, 1)
        pre_finish(0)

        pending = None
        for b in range(NB):
            qbv, kvv = views(b)
            br = blockres[b]
            ktb, kd = br["ktb"], br["kd"]

            for cc in range(BC):
                # W = K^T Qs
                pw = psum.tile([128, U * C], F32, tag="w")
                for u in range(U):
                    nc.tensor.matmul(
                        pw[:, u * C : (u + 1) * C],
                        lhsT=ktb[:, (cc * U + u) * C : (cc * U + u + 1) * C],
                        rhs=qbv[:, cc, u, :],
                        start=True,
                        stop=True,
                    )

                # state update: S <- lam^128 S + kd^T V (skip for the final
                # chunk: its updated state is never read)
                last_chunk = b == NB - 1 and cc == BC - 1
                if not last_chunk:
                    ps = psum.tile([128, U * D], F32, tag="s")
                    for u in range(U):
                        ds = slice(u * D, (u + 1) * D)
                        nc.tensor.matmul(
                            ps[:, ds],
                            lhsT=sdg_sb[:, ds],
                            rhs=s_cur[:, ds],
                            start=True,
                            stop=False,
                        )
                        nc.tensor.matmul(
                            ps[:, ds],
                            lhsT=kd[:, (cc * U + u) * D : (cc * U + u + 1) * D],
                            rhs=kvv[:, cc, u, 1, :],
                            start=False,
                            stop=True,
                        )
                    # state cast on ACT, ahead of output copies in its queue
                    s_new = state.tile([128, U * D], F16, tag="ssb")
                    nc.scalar.copy(s_new[:], ps[:])
                else:
                    s_new = s_cur

                # mask W (causal * lam^(64-j))
                wm = h16.tile([128, U * C], F16, tag="wm")
                nc.vector.tensor_tensor(
                    wm[:], pw[:], mask_sb[:], mybir.AluOpType.mult
                )

                # O-stage of the PREVIOUS chunk (one-chunk software pipeline)
                if pending is not None:
                    emit_ostage(pending)

                # next-block preamble at block end
                if cc == BC - 1 and b + 1 < NB:
                    pre_transposes(b + 1, 0)
                    pre_transposes(b + 1, 1)
                    pre_finish(b + 1)

                pending = (b, cc, wm, s_cur)
                s_cur = s_new

            if b - 1 in blockres:
                del blockres[b - 1]

        emit_ostage(pending)

    nc.compile()
    return nc


_NC_CACHE = []


def _get_nc():
    if not _NC_CACHE:
        _NC_CACHE.append(_build_nc())
    return _NC_CACHE[0]


def _core_consts(core):
    lam = _lambdas()
    i_idx = np.arange(C).astype(np.float64)
    c0v = np.zeros((128, C0_W), np.float16)
    c1v = np.zeros((128, C1_W), np.float16)
    cbv = np.zeros((128, CB_W), np.float16)
    eye = np.eye(128, dtype=np.float64)
    c0v[:, C0_ID : C0_ID + C] = np.eye(128, dtype=np.float16)
    for u in range(U):
        h = (U * core + u) % H
        l = lam[h]
        m = np.where(
            i_idx[None, :] >= i_idx[:, None],
            SCALE * l ** (i_idx[None, :] - i_idx[:, None]),
            0.0,
        )
        c0v[:, C0_MASK + u * C : C0_MASK + (u + 1) * C] = m.astype(np.float16)
        cq = (SCALE * l ** (i_idx + 1)).astype(np.float16)
        ck = (l ** (127.0 - i_idx)).astype(np.float16)
        for cc in range(BC):
            off = (cc * U + u) * C
            cbv[:, off : off + C] = np.tile(cq, (128, 1))
            c1v[:, C1_CKM + off : C1_CKM + off + C] = np.repeat(
                ck[:, None], C, axis=1
            )
        c1v[:, C1_SDG + u * C : C1_SDG + (u + 1) * C] = (l**C * eye).astype(
            np.float16
        )
    return c0v, c1v, cbv


def kernel(query_states, key_states, value_states, initial_state):
    q16 = np.asarray(query_states).astype(np.float16)
    k16 = np.asarray(key_states).astype(np.float16)
    v16 = np.asarray(value_states).astype(np.float16)
    # [B,T,H,D] -> [B*H, T, D]
    q16 = np.transpose(q16, (0, 2, 1, 3)).reshape(B * H, T, D)
    k16 = np.transpose(k16, (0, 2, 1, 3)).reshape(B * H, T, D)
    v16 = np.transpose(v16, (0, 2, 1, 3)).reshape(B * H, T, D)
    s016 = np.asarray(initial_state).astype(np.float16).reshape(B * H, D, D)

    nc = _get_nc()
    in_maps = []
    for core in range(NCORES):
        lo = U * core
        c0v, c1v, cbv = _core_consts(core)
        qs = q16[lo : lo + U]  # [U, T, D]
        ks = k16[lo : lo + U]
        vs = v16[lo : lo + U]
        # qt[b, d, (cc,u,i)]
        qb = qs.reshape(U, NB, BC, C, D).transpose(1, 4, 2, 0, 3)
        qb = np.ascontiguousarray(qb.reshape(NB, 128, BC * U * C))
        # kv[b, p, (cc,u,x,d)]
        kvb = np.stack(
            [ks.reshape(U, NB, BC, C, D), vs.reshape(U, NB, BC, C, D)], axis=4
        )  # [U, NB, BC, C, 2, D]
        kvb = kvb.transpose(1, 3, 2, 0, 4, 5)  # [NB, C, BC, U, 2, D]
        kvb = np.ascontiguousarray(kvb.reshape(NB, 128, BC * U * 2 * D))
        s0b = np.ascontiguousarray(
            s016[lo : lo + U].transpose(1, 0, 2).reshape(128, U * D)
        )
        in_maps.append(
            {
                "qt": qb,
                "kv": kvb,
                "s0": s0b,
                "cst0": c0v,
                "cst1": c1v,
                "cstb": cbv,
            }
        )

    res = run_bass_kernel_spmd(
        nc, in_maps, core_ids=list(range(NCORES)), trace=TRACE
    )
    if TRACE:
        LAST["exec_time_ns"] = res.exec_time_ns
        LAST["mean_exec_time_ns"] = res.mean_exec_time_ns
        LAST["trace"] = (
            res.instructions_and_trace[1] if res.instructions_and_trace else None
        )

    out = np.empty((B * H, T, D), np.float32)
    for core in range(NCORES):
        ob = res.results[core]["o"].reshape(NB, C, BC, U, D)
        # -> [U, NB, BC, C, D] -> [U, T, D]
        out[U * core : U * core + U] = (
            ob.transpose(3, 0, 2, 1, 4).reshape(U, T, D).astype(np.float32)
        )
    return np.ascontiguousarray(
        np.transpose(out.reshape(B, H, T, D), (0, 2, 1, 3))
    )
